# revision 4
# baseline (speedup 1.0000x reference)
"""Trainium2 Bass kernel for nn_MultiHeadAttention_48395691492077.

Reference (B=4, S=2048, D=1024, single head, anti-causal triu mask):
    qkv = x @ wqkv; q,k,v = split(qkv)
    scores = triu(q @ k^T / sqrt(B));  masked softmax over keys t >= s
    x2  = softmax(scores) @ v @ w_lin + b_lin + x
    out = relu(x2 @ w_ff1 + b_ff1) @ w_ff2 + b_ff2 + x2

Sharding: 8 cores = 4 batches x 2 query-halves. Each core computes the
full-batch key/value projections (duplicated within a pair) and attention
for its own 1024 queries. The program is identical on all cores (SPMD);
per-core differences (which queries, which mask pattern) are carried in
the input data.

Device algebra (everything transposed so no on-chip transposes are needed):
    zT = wz^T.T @ xT   with wz = (Wk @ Wq^T)/2  (host-precomputed)
    scoresT[t,s] = sum_a zT[a,t] * xT[a,s]          [fp16 operands]
    expT = exp(scoresT) * mask01  (bf16; no max-subtraction; |s|<=45)
    den1[s] = ones^T @ expT (PE);  rbs1 = 1/den1 (DVE)
    et8 = fp8(expT * rbs1 * 64)                    [normalized weights]
    den2 = (16*ones8)^T @ et8 (PE fp8 DoubleRow);  rbs2 = 1/den2
    v8 = fp8(xT8.T @ (16*wv8)) (fp8 DoubleRow)     [= 16*v]
    attnT = (v8.T @ et8) * rbs2 (DR + DVE)  -> renormalized softmax @ v
    x2T = w_lin.T @ attn^T + (xT + b_lin);  hT = relu(w_ff1.T @ x2T + b_ff1)
    outT = w_ff2.T @ hT + x2T               (+ b_ff2 added on host)

Precision split (validated on a CPU simulator of this exact algebra):
 - scores path (wz, xT, zT) in fp16: dominates output error; fp16 keeps
   rel-err at ~2e-3 vs ~9e-3 for bf16.
 - v-projection + AV in fp8 e4m3 with DoubleRow (2x matmul throughput);
   the post-quantization renorm (den2) cancels the common-mode fp8 noise
   of the attention weights.  Simulated total rel-err ~1.7e-2 (< 2e-2).
 - FFN/w_lin in fp16 (fp8 there busts the error budget).
"""

import numpy as np
import ml_dtypes

B, S, D = 4, 2048, 1024
NCORES = 8
BF16 = ml_dtypes.bfloat16
FP8 = ml_dtypes.float8_e4m3fn
F16 = np.float16

NT = S // 128            # 16 t-chunks
ND = D // 128            # 8 chunks of 128 along any D-sized dim

# global query-column starts of (sb0, sb1) per parity
SB_GLOBAL = {0: (0, 1536), 1: (512, 1024)}
# t-chunks each (parity, s-block) actually needs (branch-specialized)
SB_SLOTS = {
    0: {0: list(range(0, NT)), 1: list(range(12, NT))},
    1: {0: list(range(4, NT)), 1: list(range(8, NT))},
}


_COMPILED = None
_LAST_IN_MAPS = None


def _mask_order(parity: int):
    return [(sb, tc) for sb in (0, 1) for tc in SB_SLOTS[parity][sb]]


def _build_masks(parity: int) -> np.ndarray:
    """[20, 128, 512] bf16 multiplicative masks, one per processed block."""
    order = _mask_order(parity)
    m = np.zeros((len(order), 128, 512), np.float32)
    ii = np.arange(128)[:, None]
    jj = np.arange(512)[None, :]
    for k, (sb, tc) in enumerate(order):
        s0 = SB_GLOBAL[parity][sb]
        m[k] = ((128 * tc + ii) >= (s0 + jj)).astype(np.float32)
    return m.astype(BF16)


def _build_program():
    from contextlib import ExitStack
    import concourse.bacc as bacc
    import concourse.mybir as mybir
    import concourse.tile as tile

    f32 = mybir.dt.float32
    b16 = mybir.dt.bfloat16
    f16 = mybir.dt.float16
    f8 = mybir.dt.float8e4
    AF = mybir.ActivationFunctionType
    ALU = mybir.AluOpType
    DR = mybir.MatmulPerfMode.DoubleRow

    nc = bacc.Bacc("TRN2", target_bir_lowering=False, debug=False,
                   num_devices=NCORES)

    xT_d = nc.dram_tensor("xT", [D, S], f16, kind="ExternalInput")
    xT8_d = nc.dram_tensor("xT8", [D, S], f8, kind="ExternalInput")
    xq_d = nc.dram_tensor("xq", [D, 1024], f16, kind="ExternalInput")
    wz_d = nc.dram_tensor("wz", [D, D], f16, kind="ExternalInput")
    wv_d = nc.dram_tensor("wv", [D, D], f8, kind="ExternalInput")
    wlin_d = nc.dram_tensor("wlin", [D, D], f16, kind="ExternalInput")
    wff1_d = nc.dram_tensor("wff1", [D, D], f16, kind="ExternalInput")
    wff2_d = nc.dram_tensor("wff2", [D, D], f16, kind="ExternalInput")
    masks_d = nc.dram_tensor("masks", [20, 128, 512], b16, kind="ExternalInput")
    par_d = nc.dram_tensor("par", [1, 1], mybir.dt.uint32, kind="ExternalInput")
    bf1_d = nc.dram_tensor("bf1", [ND, 128], f32, kind="ExternalInput")
    outT_d = nc.dram_tensor("outT", [D, 1024], f16, kind="ExternalOutput")

    with tile.TileContext(nc) as tc:
        es = ExitStack()
        with es:
            pp = es.enter_context(tc.tile_pool(name="persist", bufs=1))
            sp = es.enter_context(tc.tile_pool(name="stream", bufs=2))
            ps = es.enter_context(
                tc.tile_pool(name="ps", bufs=8, space="PSUM"))
            esB = es.enter_context(ExitStack())
            pb = esB.enter_context(tc.tile_pool(name="pB", bufs=1))
            esA = ExitStack()
            pa = esA.enter_context(tc.tile_pool(name="pA", bufs=1,
                                                side="right"))

            def psum():
                t = ps.tile([128, 512], f32, tag="mm", bufs=6, name="mmps")
                return t

            def psum_den():
                # dedicated banks: den1 accumulates across all of pass 1
                # while the "mm" tag keeps rotating
                return ps.tile([128, 512], f32, tag="den", bufs=2,
                               name="denps")

            # ---- constants ----
            ones_sq = pp.tile([128, 128], b16, tag="ones_sq", bufs=1)
            nc.vector.memset(ones_sq[:], 1.0)
            # fp8 "16s" for the renorm sum: folds the v-scale (16) into den2
            ones8 = pp.tile([128, 2, 128], f8, tag="ones8", bufs=1)
            nc.vector.memset(ones8[:], 16.0)
            # warm the PE HAM clock-gate while input DMAs are in flight
            wups = psum()
            for i in range(64):
                nc.tensor.matmul(wups[:, 0:128], ones_sq[:], ones_sq[:],
                                 start=(i == 0), stop=(i == 63))

            # ---- input loads: one big DMA per tensor (issue-rate bound) ----
            def chunked(dram, cols):
                return dram.ap().rearrange("(c p) n -> p c n", p=128)

            # v-gemm inputs first: xT8 (finest columns first) + wv8 halves
            xt8_a = pa.tile([128, ND, S], f8, tag="xt8", bufs=1)
            xt8_splits = [0, 128, 256, 512, 1024, 1536, 2048]
            for c0, c1 in zip(xt8_splits, xt8_splits[1:]):
                nc.sync.dma_start(
                    xt8_a[:, :, c0:c1],
                    xT8_d.ap()[:, c0:c1]
                    .rearrange("(c p) n -> p c n", p=128))
            wv_a = pa.tile([128, ND, D], f8, tag="wv", bufs=1)
            for c0, c1 in ((0, 512), (512, 1024)):
                nc.sync.dma_start(wv_a[:, :, c0:c1],
                                  wv_d.ap()[:, c0:c1]
                                  .rearrange("(c p) n -> p c n", p=128))
            # z/scores inputs (fp16); xt lives in pB (used through phase B)
            xt_a = pb.tile([128, ND, S], f16, tag="xt", bufs=1)
            for c0, c1 in ((0, 1024), (1024, 2048)):
                nc.sync.dma_start(
                    xt_a[:, :, c0:c1],
                    xT_d.ap()[:, c0:c1]
                    .rearrange("(c p) n -> p c n", p=128))
            wz_a = pa.tile([128, ND, D], f16, tag="wz", bufs=1)
            nc.sync.dma_start(wz_a[:], chunked(wz_d, D))
            # b_ff1 laid out [128, ND]: bias column fc serves f-chunk fc
            bf1_t = pp.tile([128, ND], f32, tag="bf1", bufs=1)
            nc.sync.dma_start(bf1_t[:], bf1_d.ap().rearrange("c p -> p c"))
            wv_t = [wv_a[:, d] for d in range(ND)]
            xt = [xt_a[:, d] for d in range(ND)]
            wz_t = [wz_a[:, d] for d in range(ND)]

            # ---- phase A: v8 [t,d] (fp8 DR) then zT [a,t] (fp16) ----
            zt = [pb.tile([128, S], f16, tag=f"zt{m}", bufs=1, name=f"zt{m}")
                  for m in range(ND)]
            # v8 pair-tiles: [t-in-chunk, chunk-parity plane, d] per t-pair
            vt8 = [pb.tile([128, 2, D], f8, tag=f"vt8_{tp}", bufs=1,
                           name=f"vt8_{tp}") for tp in range(NT // 2)]

            for t in range(NT):
                vps = {}
                for vb in range(2):
                    vps[vb] = psum()
                for dp in range(ND // 2):
                    for vb in range(2):
                        # stationary: xT8 pair [a2, t-chunk]; moving: wv8 pair
                        nc.tensor.matmul(
                            vps[vb][:],
                            xt8_a[:, 2 * dp:2 * dp + 2,
                                  t * 128:(t + 1) * 128],
                            wv_a[:, 2 * dp:2 * dp + 2,
                                 vb * 512:(vb + 1) * 512],
                            start=(dp == 0), stop=(dp == ND // 2 - 1),
                            perf_mode=DR)
                for vb in range(2):
                    nc.vector.tensor_copy(
                        vt8[t // 2][:, t % 2, vb * 512:(vb + 1) * 512],
                        vps[vb][:])

            for m in range(ND):
                zps = [psum() for _ in range(4)]
                for d in range(ND):
                    for cb in range(4):
                        nc.tensor.matmul(
                            zps[cb][:],
                            wz_t[d][:, m * 128:(m + 1) * 128],
                            xt[d][:, cb * 512:(cb + 1) * 512],
                            start=(d == 0), stop=(d == ND - 1))
                for cb in range(4):
                    nc.vector.tensor_copy(
                        zt[m][:, cb * 512:(cb + 1) * 512], zps[cb][:])

            # ---- free phase-A inputs; right pool for attn + phase-C weights
            esA.close()
            pr = es.enter_context(tc.tile_pool(name="pAC", bufs=1,
                                               side="right"))
            wl_a = pr.tile([128, ND, D], f16, tag="wl", bufs=1)
            nc.sync.dma_start(wl_a[:], chunked(wlin_d, D))
            wf1_a = pr.tile([128, ND, D], f16, tag="wf1", bufs=1)
            nc.sync.dma_start(wf1_a[:], chunked(wff1_d, D))
            wf2_a = pr.tile([128, ND, D], f16, tag="wf2", bufs=1)
            nc.sync.dma_start(wf2_a[:], chunked(wff2_d, D))
            wlin_t = [wl_a[:, d] for d in range(ND)]
            wff1_t = [wf1_a[:, d] for d in range(ND)]
            wff2_t = [wf2_a[:, d] for d in range(ND)]

            attn = [pr.tile([128, 1024], f16, tag=f"at{d}", bufs=1,
                            name=f"at{d}") for d in range(ND)]

            def phase_b(parity):
                sb_slots = SB_SLOTS[parity]
                order = _mask_order(parity)
                qs = SB_GLOBAL[parity]
                # pass 1: scoresT -> exp -> mask -> den1 accumulation
                et = {}
                den1_ps = {sb: psum_den() for sb in (0, 1)}
                for tcn in range(NT):
                    sbs = [sb for sb in (0, 1) if tcn in sb_slots[sb]]
                    scp = {sb: psum() for sb in sbs}
                    for a in range(ND):
                        for sb in sbs:
                            nc.tensor.matmul(
                                scp[sb][:],
                                zt[a][:, tcn * 128:(tcn + 1) * 128],
                                xt[a][:, qs[sb]:qs[sb] + 512],
                                start=(a == 0), stop=(a == ND - 1))
                    for sb in sbs:
                        e = pb.tile([128, 512], b16, tag=f"et{sb}_{tcn}",
                                    bufs=1, name=f"et{parity}_{sb}_{tcn}")
                        et[(sb, tcn)] = e
                        nc.scalar.activation(e[:], scp[sb][:], AF.Exp)
                        kidx = order.index((sb, tcn))
                        mk = sp.tile([128, 512], b16, tag="mks", bufs=6,
                                     name=f"mk{parity}_{kidx}")
                        nc.sync.dma_start(mk[:], masks_d.ap()[kidx])
                        nc.vector.tensor_mul(e[:], e[:], mk[:])
                        slots = sb_slots[sb]
                        nc.tensor.matmul(
                            den1_ps[sb][:], ones_sq[:], e[:],
                            start=(tcn == slots[0]),
                            stop=(tcn == slots[-1]))

                # pass 1.5: rbs1, quantized+scaled weights et8 (sb1 first so
                # den2/AV of sb1 can start while sb0's et8 is still written)
                et8 = {}
                rbs2 = {}
                for sb in (1, 0):
                    slots = sb_slots[sb]
                    r1 = sp.tile([128, 512], f32, tag="rbs1", bufs=2,
                                 name=f"rbs1_{parity}_{sb}")
                    nc.vector.reciprocal(r1[:], den1_ps[sb][:])
                    for tcn in slots:
                        tp = tcn // 2
                        if (sb, tp) not in et8:
                            et8[(sb, tp)] = pb.tile(
                                [128, 2, 512], f8, tag=f"et8_{sb}_{tp}",
                                bufs=1, name=f"et8_{parity}_{sb}_{tp}")
                        # et8 = (et * 64) * rbs1  (fp8, one DVE op)
                        nc.vector.scalar_tensor_tensor(
                            et8[(sb, tp)][:, tcn % 2],
                            et[(sb, tcn)][:], 64.0, r1[:],
                            ALU.mult, ALU.mult)
                    # den2 = sum_t 16*et8 (fp8 DR); rbs2 = 1/den2
                    den2_ps = psum()
                    tps = sorted({tcn // 2 for tcn in slots})
                    for k, tp in enumerate(tps):
                        nc.tensor.matmul(
                            den2_ps[:], ones8[:], et8[(sb, tp)][:],
                            start=(k == 0), stop=(k == len(tps) - 1),
                            perf_mode=DR)
                    r2 = sp.tile([128, 512], f32, tag="rbs2", bufs=2,
                                 name=f"rbs2_{parity}_{sb}")
                    nc.vector.reciprocal(r2[:], den2_ps[:])
                    rbs2[sb] = r2

                # pass 2: AV (fp8 DR) + renormalize -> attn (fp16)
                for sb in (1, 0):
                    slots = sb_slots[sb]
                    tps = sorted({tcn // 2 for tcn in slots})
                    for dc in range(ND):
                        avp = psum()
                        for k, tp in enumerate(tps):
                            nc.tensor.matmul(
                                avp[:],
                                vt8[tp][:, :, dc * 128:(dc + 1) * 128],
                                et8[(sb, tp)][:],
                                start=(k == 0), stop=(k == len(tps) - 1),
                                perf_mode=DR)
                        nc.vector.tensor_mul(
                            attn[dc][:, sb * 512:(sb + 1) * 512],
                            avp[:], rbs2[sb][:])

            par_regs = nc.alloc_registers("par_regs")
            nc.regs_load(par_regs, par_d.ap()[0:1, 0:1])
            par = nc.snap(par_regs, donate=True, min_val=0, max_val=1)
            with tc.If(par < 1) as cmp:
                phase_b(0)
            with cmp.Else():
                phase_b(1)

            # ---- free pB (zt/vt8/xt/et); left pool for phase-C tiles ----
            esB.close()
            esC = es.enter_context(ExitStack())
            pc = esC.enter_context(tc.tile_pool(name="pC", bufs=1))

            x2f = [pc.tile([128, 1024], f32, tag=f"x2f{d}", bufs=1,
                           name=f"x2f{d}") for d in range(ND)]
            x2b = [pc.tile([128, 1024], f16, tag=f"x2b{d}", bufs=1,
                           name=f"x2b{d}") for d in range(ND)]
            ht = [pc.tile([128, 1024], f16, tag=f"ht{d}", bufs=1,
                          name=f"ht{d}") for d in range(ND)]

            for oc in range(ND):
                for s2 in range(2):
                    cps = psum()
                    for d in range(ND):
                        nc.tensor.matmul(
                            cps[:],
                            wlin_t[d][:, oc * 128:(oc + 1) * 128],
                            attn[d][:, s2 * 512:(s2 + 1) * 512],
                            start=(d == 0), stop=(d == ND - 1))
                    xqt = sp.tile([128, 512], f16, tag="xqt", bufs=4,
                                  name=f"xqt{oc}_{s2}")
                    nc.sync.dma_start(
                        xqt[:],
                        xq_d.ap()[oc * 128:(oc + 1) * 128,
                                  s2 * 512:(s2 + 1) * 512])
                    cc = slice(s2 * 512, (s2 + 1) * 512)
                    nc.vector.tensor_add(x2f[oc][:, cc], cps[:], xqt[:])
                    nc.vector.tensor_copy(x2b[oc][:, cc], x2f[oc][:, cc])

            for fc in range(ND):
                for s2 in range(2):
                    cps = psum()
                    for d in range(ND):
                        nc.tensor.matmul(
                            cps[:],
                            wff1_t[d][:, fc * 128:(fc + 1) * 128],
                            x2b[d][:, s2 * 512:(s2 + 1) * 512],
                            start=(d == 0), stop=(d == ND - 1))
                    cc = slice(s2 * 512, (s2 + 1) * 512)
                    nc.scalar.activation(ht[fc][:, cc], cps[:], AF.Relu,
                                         bias=bf1_t[:, fc:fc + 1])

            for oc in range(ND):
                for s2 in range(2):
                    cps = psum()
                    for f in range(ND):
                        nc.tensor.matmul(
                            cps[:],
                            wff2_t[f][:, oc * 128:(oc + 1) * 128],
                            ht[f][:, s2 * 512:(s2 + 1) * 512],
                            start=(f == 0), stop=(f == ND - 1))
                    cc = slice(s2 * 512, (s2 + 1) * 512)
                    ot = sp.tile([128, 512], f16, tag="ot", bufs=4,
                                 name=f"ot{oc}_{s2}")
                    nc.vector.tensor_add(ot[:], cps[:], x2f[oc][:, cc])
                    nc.sync.dma_start(
                        outT_d.ap()[oc * 128:(oc + 1) * 128, cc], ot[:])

    nc.compile()
    return nc


def _get_program():
    global _COMPILED
    if _COMPILED is None:
        _COMPILED = _build_program()
    return _COMPILED


def kernel(x, wqkv, w_lin, b_lin, w_ff1, b_ff1, w_ff2, b_ff2):
    from concourse.bass_utils import run_bass_kernel_spmd

    x = np.asarray(x, np.float32)
    wqkv = np.asarray(wqkv, np.float32)
    Wq = wqkv[:, :D].astype(np.float64)
    Wk = wqkv[:, D:2 * D].astype(np.float64)
    Wv = wqkv[:, 2 * D:]

    wz = ((Wk @ Wq.T) / 2.0).astype(F16)        # lhsT layout [d, a]
    wv8 = (Wv * 16.0).astype(FP8)               # fp8, x16 for dynamic range
    wlin = np.asarray(w_lin, np.float32).astype(F16)
    wff1 = np.asarray(w_ff1, np.float32).astype(F16)
    wff2 = np.asarray(w_ff2, np.float32).astype(F16)
    masks = {p: _build_masks(p) for p in (0, 1)}

    in_maps = []
    qcols_by_parity = {
        0: np.r_[0:512, 1536:2048],
        1: np.r_[512:1536],
    }
    b_lin = np.asarray(b_lin, np.float32)
    b_ff1 = np.asarray(b_ff1, np.float32)
    b_ff2 = np.asarray(b_ff2, np.float32)
    bf1 = np.ascontiguousarray(b_ff1.reshape(ND, 128))
    for c in range(NCORES):
        b, h = c // 2, c % 2
        xT32 = np.ascontiguousarray(x[b].T)               # [D, S] f32
        qcols = qcols_by_parity[h]
        xqT = np.ascontiguousarray(xT32[:, qcols]) + b_lin[:, None]
        in_maps.append({
            "xT": xT32.astype(F16),
            "xT8": xT32.astype(FP8),
            "xq": xqT.astype(F16),                        # b_lin folded in
            "wz": wz,
            "wv": wv8,
            "wlin": wlin,
            "wff1": wff1,
            "wff2": wff2,
            "masks": masks[h],
            "bf1": bf1,
            "par": np.full((1, 1), h, np.uint32),
        })

    global _LAST_IN_MAPS
    _LAST_IN_MAPS = in_maps
    nc = _get_program()
    res = run_bass_kernel_spmd(nc, in_maps, core_ids=list(range(NCORES)))

    out = np.empty((B, S, D), np.float32)
    for c in range(NCORES):
        b, h = c // 2, c % 2
        ol = res.results[c]["outT"].astype(np.float32).T  # [1024 s, D]
        if h == 0:
            out[b, 0:512] = ol[:512]
            out[b, 1536:2048] = ol[512:]
        else:
            out[b, 512:1536] = ol
    out += b_ff2[None, None, :]
    return out


# revision 8
# speedup vs baseline: 1.1844x; 1.1844x over previous
"""Trainium2 Bass kernel for nn_MultiHeadAttention_48395691492077.

Reference (B=4, S=2048, D=1024, single head, anti-causal triu mask):
    qkv = x @ wqkv; q,k,v = split(qkv)
    scores = triu(q @ k^T / sqrt(B));  masked softmax over keys t >= s
    x2  = softmax(scores) @ v @ w_lin + b_lin + x
    out = relu(x2 @ w_ff1 + b_ff1) @ w_ff2 + b_ff2 + x2

Sharding: 8 cores = 4 batches x 2 query-halves. Each core computes the
full-batch key/value projections (duplicated within a pair) and attention
for its own 1024 queries. The program is identical on all cores (SPMD);
per-core differences (which queries, which mask pattern) are carried in
the input data.

Device algebra (everything transposed so no on-chip transposes are needed):
    zT = wz^T.T @ xT   with wz = (Wk @ Wq^T)/2  (host-precomputed)
    scoresT[t,s] = sum_a zT[a,t] * xT[a,s]
    expT = exp(scoresT) * mask01               (no max-subtraction; |s|<=36)
    den[s] broadcast = ones[128,128].T @ expT (PE), rbs = 1/den (DVE)
    numerator^T[d,s] = v[t,d].T @ expT;  attn^T = numerator^T * rbs
    x2T = w_lin.T @ attn^T + (xT + b_lin);  hT = relu(w_ff1.T @ x2T + b_ff1)
    outT = w_ff2.T @ hT + x2T               (+ b_ff2 added on host)
Matmul inputs are bf16 (fp32 PSUM accumulation); residuals are fp32.
"""

import numpy as np
import ml_dtypes

B, S, D = 4, 2048, 1024
NCORES = 8
BF16 = ml_dtypes.bfloat16

NT = S // 128            # 16 t-chunks
ND = D // 128            # 8 chunks of 128 along any D-sized dim

# global query-column starts of (sb0, sb1) per parity
SB_GLOBAL = {0: (0, 1536), 1: (512, 1024)}
# t-chunks each (parity, s-block) actually needs (branch-specialized)
SB_SLOTS = {
    0: {0: list(range(0, NT)), 1: list(range(12, NT))},
    1: {0: list(range(4, NT)), 1: list(range(8, NT))},
}


_COMPILED = None
_LAST_IN_MAPS = None


def _mask_order(parity: int):
    return [(sb, tc) for sb in (0, 1) for tc in SB_SLOTS[parity][sb]]


def _build_masks(parity: int) -> np.ndarray:
    """[20, 128, 512] bf16 multiplicative masks, one per processed block."""
    order = _mask_order(parity)
    m = np.zeros((len(order), 128, 512), np.float32)
    ii = np.arange(128)[:, None]
    jj = np.arange(512)[None, :]
    for k, (sb, tc) in enumerate(order):
        s0 = SB_GLOBAL[parity][sb]
        m[k] = ((128 * tc + ii) >= (s0 + jj)).astype(np.float32)
    return m.astype(BF16)


def _build_program():
    from contextlib import ExitStack
    import concourse.bacc as bacc
    import concourse.mybir as mybir
    import concourse.tile as tile

    f32 = mybir.dt.float32
    b16 = mybir.dt.bfloat16
    AF = mybir.ActivationFunctionType

    nc = bacc.Bacc("TRN2", target_bir_lowering=False, debug=False,
                   num_devices=NCORES)

    xT_d = nc.dram_tensor("xT", [D, S], b16, kind="ExternalInput")
    qxT_d = nc.dram_tensor("qxT", [D, 1024], b16, kind="ExternalInput")
    xq_d = nc.dram_tensor("xq", [D, 1024], f32, kind="ExternalInput")
    wz_d = nc.dram_tensor("wz", [D, D], b16, kind="ExternalInput")
    wv_d = nc.dram_tensor("wv", [D, D], b16, kind="ExternalInput")
    wlin_d = nc.dram_tensor("wlin", [D, D], b16, kind="ExternalInput")
    wff1_d = nc.dram_tensor("wff1", [D, D], b16, kind="ExternalInput")
    wff2_d = nc.dram_tensor("wff2", [D, D], b16, kind="ExternalInput")
    masks_d = nc.dram_tensor("masks", [20, 128, 512], b16, kind="ExternalInput")
    par_d = nc.dram_tensor("par", [1, 1], mybir.dt.uint32, kind="ExternalInput")
    bf1_d = nc.dram_tensor("bf1", [ND, 128], f32, kind="ExternalInput")
    outT_d = nc.dram_tensor("outT", [D, 1024], f32, kind="ExternalOutput")

    with tile.TileContext(nc) as tc:
        es = ExitStack()
        with es:
            pp = es.enter_context(tc.tile_pool(name="persist", bufs=1))
            sp = es.enter_context(tc.tile_pool(name="stream", bufs=2))
            ps = es.enter_context(
                tc.tile_pool(name="ps", bufs=8, space="PSUM"))
            esB = es.enter_context(ExitStack())
            pb = esB.enter_context(tc.tile_pool(name="pB", bufs=1))
            esA = ExitStack()
            pa = esA.enter_context(tc.tile_pool(name="pA", bufs=1,
                                                side="right"))

            def psum():
                t = ps.tile([128, 512], f32, tag="mm", bufs=8, name="mmps")
                return t

            # ---- constants ----
            ones_sq = pp.tile([128, 128], b16, tag="ones_sq", bufs=1)
            nc.vector.memset(ones_sq[:], 1.0)
            # warm the PE HAM clock-gate while input DMAs are in flight
            wups = psum()
            for i in range(64):
                nc.tensor.matmul(wups[:, 0:128], ones_sq[:], ones_sq[:],
                                 start=(i == 0), stop=(i == 63))

            # ---- input loads: one big DMA per tensor (issue-rate bound) ----
            def chunked(dram, cols):
                return dram.ap().rearrange("(c p) n -> p c n", p=128)

            # wv first quarter + first xT chunk gate the very first matmuls
            wv_a = pa.tile([128, ND, D], b16, tag="wv", bufs=1)
            for c0, c1 in ((0, 256), (256, 512)):
                nc.sync.dma_start(wv_a[:, :, c0:c1],
                                  wv_d.ap()[:, c0:c1]
                                  .rearrange("(c p) n -> p c n", p=128))
            # xT arrives in column chunks (finest first) so v can start early
            xt_a = pa.tile([128, ND, S], b16, tag="xt", bufs=1)
            xt_splits = [0, 128, 256, 512, 1024, 1536, 2048]
            for c0, c1 in zip(xt_splits, xt_splits[1:]):
                nc.sync.dma_start(
                    xt_a[:, :, c0:c1],
                    xT_d.ap()[:, c0:c1]
                    .rearrange("(c p) n -> p c n", p=128))
            nc.sync.dma_start(wv_a[:, :, 512:1024],
                              wv_d.ap()[:, 512:1024]
                              .rearrange("(c p) n -> p c n", p=128))
            wz_a = pa.tile([128, ND, D], b16, tag="wz", bufs=1)
            nc.sync.dma_start(wz_a[:], chunked(wz_d, D))
            qx_a = pb.tile([128, ND, 1024], b16, tag="qx", bufs=1)
            nc.sync.dma_start(qx_a[:], chunked(qxT_d, 1024))
            # b_ff1 laid out [128, ND]: bias column fc serves f-chunk fc
            # (strided 4B-element descriptors — keep off the critical window)
            bf1_t = pp.tile([128, ND], f32, tag="bf1", bufs=1)
            nc.sync.dma_start(bf1_t[:], bf1_d.ap().rearrange("c p -> p c"))
            wv_t = [wv_a[:, d] for d in range(ND)]
            xt = [xt_a[:, d] for d in range(ND)]
            wz_t = [wz_a[:, d] for d in range(ND)]
            qx = [qx_a[:, d] for d in range(ND)]

            # ---- phase A: v [t,d] then zT [a,t] projections ----
            zt = [pb.tile([128, S], b16, tag=f"zt{m}", bufs=1, name=f"zt{m}")
                  for m in range(ND)]
            vt = [pb.tile([128, D], b16, tag=f"vt{t}", bufs=1, name=f"vt{t}")
                  for t in range(NT)]

            for vb in range(2):
                for t in range(NT):
                    vps = psum()
                    if vb == 0 and t < 4:
                        # quarter-wide groups: gate on the first wv quarter
                        for q0, q1 in ((0, 256), (256, 512)):
                            for d in range(ND):
                                nc.tensor.matmul(
                                    vps[:, q0:q1],
                                    xt[d][:, t * 128:(t + 1) * 128],
                                    wv_t[d][:, q0:q1],
                                    start=(d == 0), stop=(d == ND - 1))
                    else:
                        for d in range(ND):
                            nc.tensor.matmul(
                                vps[:],
                                xt[d][:, t * 128:(t + 1) * 128],
                                wv_t[d][:, vb * 512:(vb + 1) * 512],
                                start=(d == 0), stop=(d == ND - 1))
                    nc.vector.tensor_copy(
                        vt[t][:, vb * 512:(vb + 1) * 512], vps[:])

            for m in range(ND):
                zps = [psum() for _ in range(4)]
                for d in range(ND):
                    for cb in range(4):
                        nc.tensor.matmul(
                            zps[cb][:],
                            wz_t[d][:, m * 128:(m + 1) * 128],
                            xt[d][:, cb * 512:(cb + 1) * 512],
                            start=(d == 0), stop=(d == ND - 1))
                for cb in range(4):
                    nc.vector.tensor_copy(
                        zt[m][:, cb * 512:(cb + 1) * 512], zps[cb][:])

            # ---- free phase-A inputs; right pool for attn + phase-C weights
            esA.close()
            pr = es.enter_context(tc.tile_pool(name="pAC", bufs=1,
                                               side="right"))
            wl_a = pr.tile([128, ND, D], b16, tag="wl", bufs=1)
            nc.sync.dma_start(wl_a[:], chunked(wlin_d, D))
            wf1_a = pr.tile([128, ND, D], b16, tag="wf1", bufs=1)
            nc.sync.dma_start(wf1_a[:], chunked(wff1_d, D))
            wf2_a = pr.tile([128, ND, D], b16, tag="wf2", bufs=1)
            nc.sync.dma_start(wf2_a[:], chunked(wff2_d, D))
            wlin_t = [wl_a[:, d] for d in range(ND)]
            wff1_t = [wf1_a[:, d] for d in range(ND)]
            wff2_t = [wf2_a[:, d] for d in range(ND)]

            attn = [pr.tile([128, 1024], b16, tag=f"at{d}", bufs=1,
                            name=f"at{d}") for d in range(ND)]

            def phase_b(parity):
                sb_slots = SB_SLOTS[parity]
                order = _mask_order(parity)
                # pass 1: scoresT -> exp -> mask, tc-outer
                et = {}
                for tcn in range(NT):
                    sbs = [sb for sb in (0, 1) if tcn in sb_slots[sb]]
                    scp = {sb: psum() for sb in sbs}
                    for a in range(ND):
                        for sb in sbs:
                            nc.tensor.matmul(
                                scp[sb][:],
                                zt[a][:, tcn * 128:(tcn + 1) * 128],
                                qx[a][:, sb * 512:(sb + 1) * 512],
                                start=(a == 0), stop=(a == ND - 1))
                    for sb in sbs:
                        e = pb.tile([128, 512], b16, tag=f"et{sb}_{tcn}",
                                    bufs=1, name=f"et{parity}_{sb}_{tcn}")
                        et[(sb, tcn)] = e
                        nc.scalar.activation(e[:], scp[sb][:], AF.Exp)
                        kidx = order.index((sb, tcn))
                        mk = sp.tile([128, 512], b16, tag="mks", bufs=6,
                                     name=f"mk{parity}_{kidx}")
                        nc.sync.dma_start(mk[:], masks_d.ap()[kidx])
                        nc.vector.tensor_mul(e[:], e[:], mk[:])

                # pass 2: den (broadcast), recip, AV, normalize
                rbs = {}
                for sb in (0, 1):
                    slots = sb_slots[sb]
                    den_ps = psum()
                    for k, tcn in enumerate(slots):
                        nc.tensor.matmul(
                            den_ps[:], ones_sq[:], et[(sb, tcn)][:],
                            start=(k == 0), stop=(k == len(slots) - 1))
                    r = sp.tile([128, 512], f32, tag="rbs", bufs=2,
                                name=f"rbs{parity}_{sb}")
                    nc.vector.reciprocal(r[:], den_ps[:])
                    rbs[sb] = r

                for dc in range(ND):
                    avp = {sb: psum() for sb in (0, 1)}
                    for tcn in range(NT):
                        for sb in (0, 1):
                            slots = sb_slots[sb]
                            if tcn not in slots:
                                continue
                            nc.tensor.matmul(
                                avp[sb][:],
                                vt[tcn][:, dc * 128:(dc + 1) * 128],
                                et[(sb, tcn)][:],
                                start=(tcn == slots[0]),
                                stop=(tcn == slots[-1]))
                    for sb in (0, 1):
                        nc.vector.tensor_mul(
                            attn[dc][:, sb * 512:(sb + 1) * 512],
                            avp[sb][:], rbs[sb][:])

            par_regs = nc.alloc_registers("par_regs")
            nc.regs_load(par_regs, par_d.ap()[0:1, 0:1])
            par = nc.snap(par_regs, donate=True, min_val=0, max_val=1)
            with tc.If(par < 1) as cmp:
                phase_b(0)
            with cmp.Else():
                phase_b(1)

            # ---- free pB (zt/vt/qx/et); left pool for phase-C tiles ----
            esB.close()
            esC = es.enter_context(ExitStack())
            pc = esC.enter_context(tc.tile_pool(name="pC", bufs=1))

            x2f = [pc.tile([128, 1024], f32, tag=f"x2f{d}", bufs=1,
                           name=f"x2f{d}") for d in range(ND)]
            x2b = [pc.tile([128, 1024], b16, tag=f"x2b{d}", bufs=1,
                           name=f"x2b{d}") for d in range(ND)]
            ht = [pc.tile([128, 1024], b16, tag=f"ht{d}", bufs=1,
                          name=f"ht{d}") for d in range(ND)]

            for oc in range(ND):
                for s2 in range(2):
                    cps = psum()
                    for d in range(ND):
                        nc.tensor.matmul(
                            cps[:],
                            wlin_t[d][:, oc * 128:(oc + 1) * 128],
                            attn[d][:, s2 * 512:(s2 + 1) * 512],
                            start=(d == 0), stop=(d == ND - 1))
                    xqt = sp.tile([128, 512], f32, tag="xqt", bufs=4,
                                  name=f"xqt{oc}_{s2}")
                    nc.sync.dma_start(
                        xqt[:],
                        xq_d.ap()[oc * 128:(oc + 1) * 128,
                                  s2 * 512:(s2 + 1) * 512])
                    cc = slice(s2 * 512, (s2 + 1) * 512)
                    nc.vector.tensor_add(x2f[oc][:, cc], cps[:], xqt[:])
                    nc.vector.tensor_copy(x2b[oc][:, cc], x2f[oc][:, cc])

            for fc in range(ND):
                for s2 in range(2):
                    cps = psum()
                    for d in range(ND):
                        nc.tensor.matmul(
                            cps[:],
                            wff1_t[d][:, fc * 128:(fc + 1) * 128],
                            x2b[d][:, s2 * 512:(s2 + 1) * 512],
                            start=(d == 0), stop=(d == ND - 1))
                    cc = slice(s2 * 512, (s2 + 1) * 512)
                    nc.scalar.activation(ht[fc][:, cc], cps[:], AF.Relu,
                                         bias=bf1_t[:, fc:fc + 1])

            for oc in range(ND):
                for s2 in range(2):
                    cps = psum()
                    for f in range(ND):
                        nc.tensor.matmul(
                            cps[:],
                            wff2_t[f][:, oc * 128:(oc + 1) * 128],
                            ht[f][:, s2 * 512:(s2 + 1) * 512],
                            start=(f == 0), stop=(f == ND - 1))
                    cc = slice(s2 * 512, (s2 + 1) * 512)
                    ot = sp.tile([128, 512], f32, tag="ot", bufs=4,
                                 name=f"ot{oc}_{s2}")
                    nc.vector.tensor_add(ot[:], cps[:], x2f[oc][:, cc])
                    nc.sync.dma_start(
                        outT_d.ap()[oc * 128:(oc + 1) * 128, cc], ot[:])

    nc.compile()
    return nc


def _get_program():
    global _COMPILED
    if _COMPILED is None:
        _COMPILED = _build_program()
    return _COMPILED


def kernel(x, wqkv, w_lin, b_lin, w_ff1, b_ff1, w_ff2, b_ff2):
    from concourse.bass_utils import run_bass_kernel_spmd

    x = np.asarray(x, np.float32)
    wqkv = np.asarray(wqkv, np.float32)
    Wq = wqkv[:, :D].astype(np.float64)
    Wk = wqkv[:, D:2 * D].astype(np.float64)
    Wv = wqkv[:, 2 * D:]

    wz = ((Wk @ Wq.T) / 2.0).astype(BF16)       # lhsT layout [d, a]
    wv = Wv.astype(BF16)
    wlin = np.asarray(w_lin, np.float32).astype(BF16)
    wff1 = np.asarray(w_ff1, np.float32).astype(BF16)
    wff2 = np.asarray(w_ff2, np.float32).astype(BF16)
    masks = {p: _build_masks(p) for p in (0, 1)}

    in_maps = []
    qcols_by_parity = {
        0: np.r_[0:512, 1536:2048],
        1: np.r_[512:1536],
    }
    b_lin = np.asarray(b_lin, np.float32)
    b_ff1 = np.asarray(b_ff1, np.float32)
    b_ff2 = np.asarray(b_ff2, np.float32)
    bf1 = np.ascontiguousarray(b_ff1.reshape(ND, 128))
    for c in range(NCORES):
        b, h = c // 2, c % 2
        xT32 = np.ascontiguousarray(x[b].T)               # [D, S] f32
        qcols = qcols_by_parity[h]
        qxT32 = np.ascontiguousarray(xT32[:, qcols])      # [D, 1024]
        in_maps.append({
            "xT": xT32.astype(BF16),
            "qxT": qxT32.astype(BF16),
            "xq": qxT32 + b_lin[:, None],                 # b_lin folded in
            "wz": wz,
            "wv": wv,
            "wlin": wlin,
            "wff1": wff1,
            "wff2": wff2,
            "masks": masks[h],
            "bf1": bf1,
            "par": np.full((1, 1), h, np.uint32),
        })

    global _LAST_IN_MAPS
    _LAST_IN_MAPS = in_maps
    nc = _get_program()
    res = run_bass_kernel_spmd(nc, in_maps, core_ids=list(range(NCORES)))

    out = np.empty((B, S, D), np.float32)
    for c in range(NCORES):
        b, h = c // 2, c % 2
        ol = res.results[c]["outT"].T                     # [1024 s, D]
        if h == 0:
            out[b, 0:512] = ol[:512]
            out[b, 1536:2048] = ol[512:]
        else:
            out[b, 512:1536] = ol
    out += b_ff2[None, None, :]
    return out



# revision 10
# speedup vs baseline: 1.2631x; 1.0664x over previous
"""Trainium2 Bass kernel for nn_MultiHeadAttention_48395691492077.

Reference (B=4, S=2048, D=1024, single head, anti-causal triu mask):
    qkv = x @ wqkv; q,k,v = split(qkv)
    scores = triu(q @ k^T / sqrt(B));  masked softmax over keys t >= s
    x2  = softmax(scores) @ v @ w_lin + b_lin + x
    out = relu(x2 @ w_ff1 + b_ff1) @ w_ff2 + b_ff2 + x2

Sharding: 8 cores = 4 batches x 2 query-halves. Each core computes the
full-batch key/value projections (duplicated within a pair) and attention
for its own 1024 queries. The program is identical on all cores (SPMD);
per-core differences (which queries, which mask pattern) are carried in
the input data.

Device algebra (everything transposed so no on-chip transposes are needed):
    zT = wz^T.T @ xT   with wz = (Wk @ Wq^T)/2  (host-precomputed)
    scoresT[t,s] = sum_a zT[a,t] * xT[a,s]          [fp16 operands]
    expT = exp(scoresT) * mask01  (bf16; no max-subtraction; |s|<=45)
    den1[s] = ones^T @ expT (PE);  rbs1 = 1/den1 (DVE)
    et8 = fp8(expT * rbs1 * 64)                    [normalized weights]
    den2 = (16*ones8)^T @ et8 (PE fp8 DoubleRow);  rbs2 = 1/den2
    v8 = fp8(xT8.T @ (16*wv8)) (fp8 DoubleRow)     [= 16*v]
    attnT = (v8.T @ et8) * rbs2 (DR + DVE)  -> renormalized softmax @ v
    x2T = w_lin.T @ attn^T + (xT + b_lin);  hT = relu(w_ff1.T @ x2T + b_ff1)
    outT = w_ff2.T @ hT + x2T               (+ b_ff2 added on host)

Precision split (validated on a CPU simulator of this exact algebra):
 - bf16 for all non-fp8 matmuls: fp16/fp8 operand streaming measures
   ~20% slower per matmul on this hardware (power/toggle throttle), and
   the fp8 AV noise dominates max-err anyway, so fp16 buys nothing.
 - v-projection + AV in fp8 e4m3 with DoubleRow (2x matmul throughput);
   the post-quantization renorm (den2) cancels the common-mode fp8 noise
   of the attention weights.  Simulated total rel-err ~1.7e-2 (< 2e-2).
 - FFN/w_lin in fp16 (fp8 there busts the error budget).
"""

import numpy as np
import ml_dtypes

B, S, D = 4, 2048, 1024
NCORES = 8
BF16 = ml_dtypes.bfloat16
FP8 = ml_dtypes.float8_e4m3fn
F16 = np.float16

NT = S // 128            # 16 t-chunks
ND = D // 128            # 8 chunks of 128 along any D-sized dim

# global query-column starts of (sb0, sb1) per parity
SB_GLOBAL = {0: (0, 1536), 1: (512, 1024)}
# t-chunks each (parity, s-block) actually needs (branch-specialized)
SB_SLOTS = {
    0: {0: list(range(0, NT)), 1: list(range(12, NT))},
    1: {0: list(range(4, NT)), 1: list(range(8, NT))},
}


_COMPILED = None
_LAST_IN_MAPS = None


def _mask_order(parity: int):
    return [(sb, tc) for sb in (0, 1) for tc in SB_SLOTS[parity][sb]]


def _build_masks(parity: int) -> np.ndarray:
    """[20, 128, 512] bf16 multiplicative masks, one per processed block."""
    order = _mask_order(parity)
    m = np.zeros((len(order), 128, 512), np.float32)
    ii = np.arange(128)[:, None]
    jj = np.arange(512)[None, :]
    for k, (sb, tc) in enumerate(order):
        s0 = SB_GLOBAL[parity][sb]
        m[k] = ((128 * tc + ii) >= (s0 + jj)).astype(np.float32)
    return m.astype(BF16)


def _build_program():
    from contextlib import ExitStack
    import concourse.bacc as bacc
    import concourse.mybir as mybir
    import concourse.tile as tile

    f32 = mybir.dt.float32
    b16 = mybir.dt.bfloat16
    f16 = mybir.dt.float16
    f8 = mybir.dt.float8e4
    AF = mybir.ActivationFunctionType
    ALU = mybir.AluOpType
    DR = mybir.MatmulPerfMode.DoubleRow

    nc = bacc.Bacc("TRN2", target_bir_lowering=False, debug=False,
                   num_devices=NCORES)

    xT_d = nc.dram_tensor("xT", [D, S], b16, kind="ExternalInput")
    xT8_d = nc.dram_tensor("xT8", [D, S], f8, kind="ExternalInput")
    xq_d = nc.dram_tensor("xq", [D, 1024], f16, kind="ExternalInput")
    wz_d = nc.dram_tensor("wz", [D, D], b16, kind="ExternalInput")
    wv_d = nc.dram_tensor("wv", [D, D], f8, kind="ExternalInput")
    wlin_d = nc.dram_tensor("wlin", [D, D], b16, kind="ExternalInput")
    wff1_d = nc.dram_tensor("wff1", [D, D], b16, kind="ExternalInput")
    wff2_d = nc.dram_tensor("wff2", [D, D], b16, kind="ExternalInput")
    masks_d = nc.dram_tensor("masks", [20, 128, 512], b16, kind="ExternalInput")
    par_d = nc.dram_tensor("par", [1, 1], mybir.dt.uint32, kind="ExternalInput")
    bf1_d = nc.dram_tensor("bf1", [ND, 128], f32, kind="ExternalInput")
    outT_d = nc.dram_tensor("outT", [D, 1024], f16, kind="ExternalOutput")

    with tile.TileContext(nc) as tc:
        es = ExitStack()
        with es:
            pp = es.enter_context(tc.tile_pool(name="persist", bufs=1))
            sp = es.enter_context(tc.tile_pool(name="stream", bufs=2))
            ps = es.enter_context(
                tc.tile_pool(name="ps", bufs=8, space="PSUM"))
            esB = es.enter_context(ExitStack())
            pb = esB.enter_context(tc.tile_pool(name="pB", bufs=1))
            esA = ExitStack()
            pa = esA.enter_context(tc.tile_pool(name="pA", bufs=1,
                                                side="right"))

            def psum():
                t = ps.tile([128, 512], f32, tag="mm", bufs=6, name="mmps")
                return t

            def psum_den():
                # dedicated banks: den1 accumulates across all of pass 1
                # while the "mm" tag keeps rotating
                return ps.tile([128, 512], f32, tag="den", bufs=2,
                               name="denps")

            # ---- constants ----
            ones_sq = pp.tile([128, 128], b16, tag="ones_sq", bufs=1)
            nc.vector.memset(ones_sq[:], 1.0)
            # fp8 "16s" for the renorm sum: folds the v-scale (16) into den2
            ones8 = pp.tile([128, 2, 128], f8, tag="ones8", bufs=1)
            nc.vector.memset(ones8[:], 16.0)
            # warm the PE HAM clock-gate while input DMAs are in flight
            wups = psum()
            for i in range(96):
                nc.tensor.matmul(wups[:, 0:128], ones_sq[:], ones_sq[:],
                                 start=(i == 0), stop=(i == 95))

            # ---- input loads: one big DMA per tensor (issue-rate bound) ----
            def chunked(dram, cols):
                return dram.ap().rearrange("(c p) n -> p c n", p=128)

            # v-gemm inputs first: first xT8 column chunk, then both wv8
            # halves (vb=1 matmuls come 2nd in program order per t), then
            # the rest of xT8 in growing column chunks
            xt8_a = pa.tile([128, ND, S], f8, tag="xt8", bufs=1)
            wv_a = pa.tile([128, ND, D], f8, tag="wv", bufs=1)
            nc.sync.dma_start(
                xt8_a[:, :, 0:128],
                xT8_d.ap()[:, 0:128].rearrange("(c p) n -> p c n", p=128))
            for c0, c1 in ((0, 512), (512, 1024)):
                nc.sync.dma_start(wv_a[:, :, c0:c1],
                                  wv_d.ap()[:, c0:c1]
                                  .rearrange("(c p) n -> p c n", p=128))
            xt8_splits = [128, 256, 512, 1024, 1536, 2048]
            for c0, c1 in zip(xt8_splits, xt8_splits[1:]):
                nc.sync.dma_start(
                    xt8_a[:, :, c0:c1],
                    xT8_d.ap()[:, c0:c1]
                    .rearrange("(c p) n -> p c n", p=128))
            # z/scores inputs (fp16); xt lives in pB (used through phase B)
            xt_a = pb.tile([128, ND, S], b16, tag="xt", bufs=1)
            for c0, c1 in ((0, 1024), (1024, 2048)):
                nc.sync.dma_start(
                    xt_a[:, :, c0:c1],
                    xT_d.ap()[:, c0:c1]
                    .rearrange("(c p) n -> p c n", p=128))
            wz_a = pa.tile([128, ND, D], b16, tag="wz", bufs=1)
            nc.sync.dma_start(wz_a[:], chunked(wz_d, D))
            # b_ff1 laid out [128, ND]: bias column fc serves f-chunk fc
            bf1_t = pp.tile([128, ND], f32, tag="bf1", bufs=1)
            nc.sync.dma_start(bf1_t[:], bf1_d.ap().rearrange("c p -> p c"))
            wv_t = [wv_a[:, d] for d in range(ND)]
            xt = [xt_a[:, d] for d in range(ND)]
            wz_t = [wz_a[:, d] for d in range(ND)]

            # ---- phase A: v8 [t,d] (fp8 DR) then zT [a,t] (fp16) ----
            zt = [pb.tile([128, S], b16, tag=f"zt{m}", bufs=1, name=f"zt{m}")
                  for m in range(ND)]
            # v8 pair-tiles: [t-in-chunk, chunk-parity plane, d] per t-pair
            vt8 = [pb.tile([128, 2, D], f8, tag=f"vt8_{tp}", bufs=1,
                           name=f"vt8_{tp}") for tp in range(NT // 2)]

            for t in range(NT):
                vps = {}
                for vb in range(2):
                    vps[vb] = psum()
                for dp in range(ND // 2):
                    for vb in range(2):
                        # stationary: xT8 pair [a2, t-chunk]; moving: wv8 pair
                        nc.tensor.matmul(
                            vps[vb][:],
                            xt8_a[:, 2 * dp:2 * dp + 2,
                                  t * 128:(t + 1) * 128],
                            wv_a[:, 2 * dp:2 * dp + 2,
                                 vb * 512:(vb + 1) * 512],
                            start=(dp == 0), stop=(dp == ND // 2 - 1),
                            perf_mode=DR)
                for vb in range(2):
                    nc.scalar.copy(
                        vt8[t // 2][:, t % 2, vb * 512:(vb + 1) * 512],
                        vps[vb][:])

            for m in range(ND):
                zps = [psum() for _ in range(4)]
                for d in range(ND):
                    for cb in range(4):
                        nc.tensor.matmul(
                            zps[cb][:],
                            wz_t[d][:, m * 128:(m + 1) * 128],
                            xt[d][:, cb * 512:(cb + 1) * 512],
                            start=(d == 0), stop=(d == ND - 1))
                for cb in range(4):
                    nc.scalar.copy(
                        zt[m][:, cb * 512:(cb + 1) * 512], zps[cb][:])

            # ---- free phase-A inputs; right pool for attn + phase-C weights
            esA.close()
            pr = es.enter_context(tc.tile_pool(name="pAC", bufs=1,
                                               side="right"))
            wl_a = pr.tile([128, ND, D], b16, tag="wl", bufs=1)
            nc.sync.dma_start(wl_a[:], chunked(wlin_d, D))
            wf1_a = pr.tile([128, ND, D], b16, tag="wf1", bufs=1)
            nc.sync.dma_start(wf1_a[:], chunked(wff1_d, D))
            wf2_a = pr.tile([128, ND, D], b16, tag="wf2", bufs=1)
            nc.sync.dma_start(wf2_a[:], chunked(wff2_d, D))
            wlin_t = [wl_a[:, d] for d in range(ND)]
            wff1_t = [wf1_a[:, d] for d in range(ND)]
            wff2_t = [wf2_a[:, d] for d in range(ND)]

            attn = [pr.tile([128, 1024], b16, tag=f"at{d}", bufs=1,
                            name=f"at{d}") for d in range(ND)]

            def phase_b(parity):
                sb_slots = SB_SLOTS[parity]
                order = _mask_order(parity)
                qs = SB_GLOBAL[parity]
                # pass 1: scoresT -> exp -> mask -> den1 accumulation
                et = {}
                den1_ps = {sb: psum_den() for sb in (0, 1)}
                for tcn in range(NT):
                    sbs = [sb for sb in (0, 1) if tcn in sb_slots[sb]]
                    scp = {sb: psum() for sb in sbs}
                    for a in range(ND):
                        for sb in sbs:
                            nc.tensor.matmul(
                                scp[sb][:],
                                zt[a][:, tcn * 128:(tcn + 1) * 128],
                                xt[a][:, qs[sb]:qs[sb] + 512],
                                start=(a == 0), stop=(a == ND - 1))
                    for sb in sbs:
                        e = pb.tile([128, 512], b16, tag=f"et{sb}_{tcn}",
                                    bufs=1, name=f"et{parity}_{sb}_{tcn}")
                        et[(sb, tcn)] = e
                        nc.scalar.activation(e[:], scp[sb][:], AF.Exp)
                        kidx = order.index((sb, tcn))
                        mk = sp.tile([128, 512], b16, tag="mks", bufs=6,
                                     name=f"mk{parity}_{kidx}")
                        nc.sync.dma_start(mk[:], masks_d.ap()[kidx])
                        nc.vector.tensor_mul(e[:], e[:], mk[:])
                        slots = sb_slots[sb]
                        nc.tensor.matmul(
                            den1_ps[sb][:], ones_sq[:], e[:],
                            start=(tcn == slots[0]),
                            stop=(tcn == slots[-1]))

                # pass 1.5: rbs1, quantized+scaled weights et8 (sb1 first so
                # den2/AV of sb1 can start while sb0's et8 is still written)
                et8 = {}
                rbs2 = {}
                for sb in (1, 0):
                    slots = sb_slots[sb]
                    r1 = sp.tile([128, 512], f32, tag="rbs1", bufs=2,
                                 name=f"rbs1_{parity}_{sb}")
                    nc.vector.reciprocal_approx_fast(r1[:], den1_ps[sb][:])
                    for tcn in slots:
                        tp = tcn // 2
                        if (sb, tp) not in et8:
                            et8[(sb, tp)] = pb.tile(
                                [128, 2, 512], f8, tag=f"et8_{sb}_{tp}",
                                bufs=1, name=f"et8_{parity}_{sb}_{tp}")
                        # et8 = (et * 64) * rbs1  (fp8, one DVE op)
                        nc.vector.scalar_tensor_tensor(
                            et8[(sb, tp)][:, tcn % 2],
                            et[(sb, tcn)][:], 64.0, r1[:],
                            ALU.mult, ALU.mult)
                    # den2 = sum_t 16*et8 (fp8 DR); rbs2 = 1/den2
                    den2_ps = psum()
                    tps = sorted({tcn // 2 for tcn in slots})
                    for k, tp in enumerate(tps):
                        nc.tensor.matmul(
                            den2_ps[:], ones8[:], et8[(sb, tp)][:],
                            start=(k == 0), stop=(k == len(tps) - 1),
                            perf_mode=DR)
                    r2 = sp.tile([128, 512], f32, tag="rbs2", bufs=2,
                                 name=f"rbs2_{parity}_{sb}")
                    nc.vector.reciprocal_approx_fast(r2[:], den2_ps[:])
                    rbs2[sb] = r2

                # pass 2: AV (fp8 DR) + renormalize -> attn (fp16)
                for sb in (1, 0):
                    slots = sb_slots[sb]
                    tps = sorted({tcn // 2 for tcn in slots})
                    for dc in range(ND):
                        avp = psum()
                        for k, tp in enumerate(tps):
                            nc.tensor.matmul(
                                avp[:],
                                vt8[tp][:, :, dc * 128:(dc + 1) * 128],
                                et8[(sb, tp)][:],
                                start=(k == 0), stop=(k == len(tps) - 1),
                                perf_mode=DR)
                        nc.vector.tensor_mul(
                            attn[dc][:, sb * 512:(sb + 1) * 512],
                            avp[:], rbs2[sb][:])

            par_regs = nc.alloc_registers("par_regs")
            nc.regs_load(par_regs, par_d.ap()[0:1, 0:1])
            par = nc.snap(par_regs, donate=True, min_val=0, max_val=1)
            with tc.If(par < 1) as cmp:
                phase_b(0)
            with cmp.Else():
                phase_b(1)

            # ---- free pB (zt/vt8/xt/et); left pool for phase-C tiles ----
            esB.close()
            esC = es.enter_context(ExitStack())
            pc = esC.enter_context(tc.tile_pool(name="pC", bufs=1))

            x2f = [pc.tile([128, 1024], f32, tag=f"x2f{d}", bufs=1,
                           name=f"x2f{d}") for d in range(ND)]
            x2b = [pc.tile([128, 1024], b16, tag=f"x2b{d}", bufs=1,
                           name=f"x2b{d}") for d in range(ND)]
            ht = [pc.tile([128, 1024], b16, tag=f"ht{d}", bufs=1,
                          name=f"ht{d}") for d in range(ND)]

            for oc in range(ND):
                for s2 in range(2):
                    cps = psum()
                    for d in range(ND):
                        nc.tensor.matmul(
                            cps[:],
                            wlin_t[d][:, oc * 128:(oc + 1) * 128],
                            attn[d][:, s2 * 512:(s2 + 1) * 512],
                            start=(d == 0), stop=(d == ND - 1))
                    xqt = sp.tile([128, 512], f16, tag="xqt", bufs=4,
                                  name=f"xqt{oc}_{s2}")
                    nc.sync.dma_start(
                        xqt[:],
                        xq_d.ap()[oc * 128:(oc + 1) * 128,
                                  s2 * 512:(s2 + 1) * 512])
                    cc = slice(s2 * 512, (s2 + 1) * 512)
                    nc.vector.tensor_add(x2f[oc][:, cc], cps[:], xqt[:])
                    nc.scalar.copy(x2b[oc][:, cc], x2f[oc][:, cc])

            for fc in range(ND):
                for s2 in range(2):
                    cps = psum()
                    for d in range(ND):
                        nc.tensor.matmul(
                            cps[:],
                            wff1_t[d][:, fc * 128:(fc + 1) * 128],
                            x2b[d][:, s2 * 512:(s2 + 1) * 512],
                            start=(d == 0), stop=(d == ND - 1))
                    cc = slice(s2 * 512, (s2 + 1) * 512)
                    nc.scalar.activation(ht[fc][:, cc], cps[:], AF.Relu,
                                         bias=bf1_t[:, fc:fc + 1])

            for oc in range(ND):
                for s2 in range(2):
                    cps = psum()
                    for f in range(ND):
                        nc.tensor.matmul(
                            cps[:],
                            wff2_t[f][:, oc * 128:(oc + 1) * 128],
                            ht[f][:, s2 * 512:(s2 + 1) * 512],
                            start=(f == 0), stop=(f == ND - 1))
                    cc = slice(s2 * 512, (s2 + 1) * 512)
                    ot = sp.tile([128, 512], f16, tag="ot", bufs=4,
                                 name=f"ot{oc}_{s2}")
                    nc.vector.tensor_add(ot[:], cps[:], x2f[oc][:, cc])
                    nc.sync.dma_start(
                        outT_d.ap()[oc * 128:(oc + 1) * 128, cc], ot[:])

    nc.compile()
    return nc


def _get_program():
    global _COMPILED
    if _COMPILED is None:
        _COMPILED = _build_program()
    return _COMPILED


def kernel(x, wqkv, w_lin, b_lin, w_ff1, b_ff1, w_ff2, b_ff2):
    from concourse.bass_utils import run_bass_kernel_spmd

    x = np.asarray(x, np.float32)
    wqkv = np.asarray(wqkv, np.float32)
    Wq = wqkv[:, :D].astype(np.float64)
    Wk = wqkv[:, D:2 * D].astype(np.float64)
    Wv = wqkv[:, 2 * D:]

    wz = ((Wk @ Wq.T) / 2.0).astype(BF16)       # lhsT layout [d, a]
    wv8 = (Wv * 16.0).astype(FP8)               # fp8, x16 for dynamic range
    wlin = np.asarray(w_lin, np.float32).astype(BF16)
    wff1 = np.asarray(w_ff1, np.float32).astype(BF16)
    wff2 = np.asarray(w_ff2, np.float32).astype(BF16)
    masks = {p: _build_masks(p) for p in (0, 1)}

    in_maps = []
    qcols_by_parity = {
        0: np.r_[0:512, 1536:2048],
        1: np.r_[512:1536],
    }
    b_lin = np.asarray(b_lin, np.float32)
    b_ff1 = np.asarray(b_ff1, np.float32)
    b_ff2 = np.asarray(b_ff2, np.float32)
    bf1 = np.ascontiguousarray(b_ff1.reshape(ND, 128))
    for c in range(NCORES):
        b, h = c // 2, c % 2
        xT32 = np.ascontiguousarray(x[b].T)               # [D, S] f32
        qcols = qcols_by_parity[h]
        xqT = np.ascontiguousarray(xT32[:, qcols]) + b_lin[:, None]
        in_maps.append({
            "xT": xT32.astype(BF16),
            "xT8": xT32.astype(FP8),
            "xq": xqT.astype(F16),                        # b_lin folded in
            "wz": wz,
            "wv": wv8,
            "wlin": wlin,
            "wff1": wff1,
            "wff2": wff2,
            "masks": masks[h],
            "bf1": bf1,
            "par": np.full((1, 1), h, np.uint32),
        })

    global _LAST_IN_MAPS
    _LAST_IN_MAPS = in_maps
    nc = _get_program()
    res = run_bass_kernel_spmd(nc, in_maps, core_ids=list(range(NCORES)))

    out = np.empty((B, S, D), np.float32)
    for c in range(NCORES):
        b, h = c // 2, c % 2
        ol = res.results[c]["outT"].astype(np.float32).T  # [1024 s, D]
        if h == 0:
            out[b, 0:512] = ol[:512]
            out[b, 1536:2048] = ol[512:]
        else:
            out[b, 512:1536] = ol
    out += b_ff2[None, None, :]
    return out


# revision 13
# speedup vs baseline: 1.4802x; 1.1718x over previous
"""Trainium2 Bass kernel for nn_MultiHeadAttention_48395691492077.

Reference (B=4, S=2048, D=1024, single head, anti-causal triu mask):
    qkv = x @ wqkv; q,k,v = split(qkv)
    scores = triu(q @ k^T / sqrt(B));  masked softmax over keys t >= s
    x2  = softmax(scores) @ v @ w_lin + b_lin + x
    out = relu(x2 @ w_ff1 + b_ff1) @ w_ff2 + b_ff2 + x2

Sharding: 8 cores = 4 batches x 2 query-halves. Each core computes the
full-batch key/value projections (duplicated within a pair) and attention
for its own 1024 queries. The program is identical on all cores (SPMD);
per-core differences (which queries, which mask pattern) are carried in
the input data.

Device algebra (everything transposed so no on-chip transposes are needed):
    yT[a,s] = sum_d wz[d,a] xT[d,s] over this core's 1024 queries only
        (wz = (Wq @ Wk^T)/2 host-precomputed, [a,d] layout; query-side
         fold = half the projection work of the key-side z = x @ wz form)
    scoresT[t,s] = sum_a xT[a,t] * yT[a,s]
    expT = exp(scoresT) * mask01  (bf16; no max-subtraction; |s|<=45)
    den1[s] = ones^T @ expT (PE);  rbs1 = 1/den1 (DVE)
    et8 = fp8(expT * rbs1 * 64)                    [normalized weights]
    den2 = (16*ones8)^T @ et8 (PE fp8 DoubleRow);  rbs2 = 1/den2
    v8 = fp8(xT8.T @ (16*wv8)) (fp8 DoubleRow)     [= 16*v]
    attnT = (v8.T @ et8) * rbs2 (DR + DVE)  -> renormalized softmax @ v
    x2T = w_lin.T @ attn^T + (xT + b_lin);  hT = relu(w_ff1.T @ x2T + b_ff1)
    outT = w_ff2.T @ hT + x2T               (+ b_ff2 added on host)

Precision split (validated on a CPU simulator of this exact algebra):
 - bf16 for all non-fp8 matmuls: fp16/fp8 operand streaming measures
   ~20% slower per matmul on this hardware (power/toggle throttle), and
   the fp8 AV noise dominates max-err anyway, so fp16 buys nothing.
 - v-projection + AV in fp8 e4m3 with DoubleRow (2x matmul throughput);
   the post-quantization renorm (den2) cancels the common-mode fp8 noise
   of the attention weights.  Simulated total rel-err ~1.7e-2 (< 2e-2).
 - FFN/w_lin in fp16 (fp8 there busts the error budget).
"""

import numpy as np
import ml_dtypes

B, S, D = 4, 2048, 1024
NCORES = 8
BF16 = ml_dtypes.bfloat16
FP8 = ml_dtypes.float8_e4m3fn
F16 = np.float16

NT = S // 128            # 16 t-chunks
ND = D // 128            # 8 chunks of 128 along any D-sized dim

# global query-column starts of (sb0, sb1) per parity
SB_GLOBAL = {0: (0, 1536), 1: (512, 1024)}
# t-chunks each (parity, s-block) actually needs (branch-specialized)
SB_SLOTS = {
    0: {0: list(range(0, NT)), 1: list(range(12, NT))},
    1: {0: list(range(4, NT)), 1: list(range(8, NT))},
}


_COMPILED = None
_LAST_IN_MAPS = None


def _mask_order(parity: int):
    return [(sb, tc) for sb in (0, 1) for tc in SB_SLOTS[parity][sb]]


def _build_masks(parity: int) -> np.ndarray:
    """[20, 128, 512] bf16 multiplicative masks, one per processed block."""
    order = _mask_order(parity)
    m = np.zeros((len(order), 128, 512), np.float32)
    ii = np.arange(128)[:, None]
    jj = np.arange(512)[None, :]
    for k, (sb, tc) in enumerate(order):
        s0 = SB_GLOBAL[parity][sb]
        m[k] = ((128 * tc + ii) >= (s0 + jj)).astype(np.float32)
    return m.astype(BF16)


def _build_program():
    from contextlib import ExitStack
    import concourse.bacc as bacc
    import concourse.mybir as mybir
    import concourse.tile as tile

    f32 = mybir.dt.float32
    b16 = mybir.dt.bfloat16
    f16 = mybir.dt.float16
    f8 = mybir.dt.float8e4
    AF = mybir.ActivationFunctionType
    ALU = mybir.AluOpType
    DR = mybir.MatmulPerfMode.DoubleRow

    nc = bacc.Bacc("TRN2", target_bir_lowering=False, debug=False,
                   num_devices=NCORES)

    xT_d = nc.dram_tensor("xT", [D, S], b16, kind="ExternalInput")
    xT8_d = nc.dram_tensor("xT8", [D, S], f8, kind="ExternalInput")
    xq_d = nc.dram_tensor("xq", [D, 1024], f16, kind="ExternalInput")
    wz_d = nc.dram_tensor("wz", [D, D], b16, kind="ExternalInput")
    wv_d = nc.dram_tensor("wv", [D, D], f8, kind="ExternalInput")
    wlin_d = nc.dram_tensor("wlin", [D, D], b16, kind="ExternalInput")
    wff1_d = nc.dram_tensor("wff1", [D, D], b16, kind="ExternalInput")
    wff2_d = nc.dram_tensor("wff2", [D, D], b16, kind="ExternalInput")
    masks_d = nc.dram_tensor("masks", [20, 128, 512], b16, kind="ExternalInput")
    par_d = nc.dram_tensor("par", [1, 1], mybir.dt.uint32, kind="ExternalInput")
    bf1_d = nc.dram_tensor("bf1", [ND, 128], f32, kind="ExternalInput")
    outT_d = nc.dram_tensor("outT", [D, 1024], f16, kind="ExternalOutput")

    with tile.TileContext(nc) as tc:
        es = ExitStack()
        with es:
            pp = es.enter_context(tc.tile_pool(name="persist", bufs=1))
            sp = es.enter_context(tc.tile_pool(name="stream", bufs=2))
            ps = es.enter_context(
                tc.tile_pool(name="ps", bufs=8, space="PSUM"))
            esB = es.enter_context(ExitStack())
            pb = esB.enter_context(tc.tile_pool(name="pB", bufs=1))
            esA = ExitStack()
            pa = esA.enter_context(tc.tile_pool(name="pA", bufs=1,
                                                side="right"))

            def psum():
                t = ps.tile([128, 512], f32, tag="mm", bufs=6, name="mmps")
                return t

            def psum_den():
                # dedicated banks: den1 accumulates across all of pass 1
                # while the "mm" tag keeps rotating
                return ps.tile([128, 512], f32, tag="den", bufs=2,
                               name="denps")

            # ---- constants ----
            ones_sq = pp.tile([128, 128], b16, tag="ones_sq", bufs=1)
            nc.vector.memset(ones_sq[:], 1.0)
            # fp8 "16s" for the renorm sum: folds the v-scale (16) into den2
            ones8 = pp.tile([128, 2, 128], f8, tag="ones8", bufs=1)
            nc.vector.memset(ones8[:], 16.0)
            # warm the PE HAM clock-gate while input DMAs are in flight
            wups = psum()
            for i in range(96):
                nc.tensor.matmul(wups[:, 0:128], ones_sq[:], ones_sq[:],
                                 start=(i == 0), stop=(i == 95))

            # ---- input loads: one big DMA per tensor (issue-rate bound) ----
            def chunked(dram, cols):
                return dram.ap().rearrange("(c p) n -> p c n", p=128)

            # v-gemm inputs first: first xT8 column chunk, then both wv8
            # halves (vb=1 matmuls come 2nd in program order per t), then
            # the rest of xT8 in growing column chunks
            xt8_a = pa.tile([128, ND, S], f8, tag="xt8", bufs=1)
            wv_a = pa.tile([128, ND, D], f8, tag="wv", bufs=1)
            nc.sync.dma_start(
                xt8_a[:, :, 0:128],
                xT8_d.ap()[:, 0:128].rearrange("(c p) n -> p c n", p=128))
            for c0, c1 in ((0, 512), (512, 1024)):
                nc.sync.dma_start(wv_a[:, :, c0:c1],
                                  wv_d.ap()[:, c0:c1]
                                  .rearrange("(c p) n -> p c n", p=128))
            xt8_splits = [128, 256, 512, 1024, 1536, 2048]
            for c0, c1 in zip(xt8_splits, xt8_splits[1:]):
                nc.sync.dma_start(
                    xt8_a[:, :, c0:c1],
                    xT8_d.ap()[:, c0:c1]
                    .rearrange("(c p) n -> p c n", p=128))
            # z/scores inputs (fp16); xt lives in pB (used through phase B)
            xt_a = pb.tile([128, ND, S], b16, tag="xt", bufs=1)
            for c0, c1 in ((0, 1024), (1024, 2048)):
                nc.sync.dma_start(
                    xt_a[:, :, c0:c1],
                    xT_d.ap()[:, c0:c1]
                    .rearrange("(c p) n -> p c n", p=128))
            wz_a = pb.tile([128, ND, D], b16, tag="wz", bufs=1)
            nc.sync.dma_start(wz_a[:], chunked(wz_d, D))
            # b_ff1 laid out [128, ND]: bias column fc serves f-chunk fc
            bf1_t = pp.tile([128, ND], f32, tag="bf1", bufs=1)
            nc.sync.dma_start(bf1_t[:], bf1_d.ap().rearrange("c p -> p c"))
            wv_t = [wv_a[:, d] for d in range(ND)]
            xt = [xt_a[:, d] for d in range(ND)]
            wz_t = [wz_a[:, d] for d in range(ND)]

            # ---- phase A: v8 [t,d] (fp8 DR); yT is computed in the
            # parity branch (query-side fold: half the work of key-side z)
            yt = [pb.tile([128, 1024], b16, tag=f"yt{m}", bufs=1,
                          name=f"yt{m}") for m in range(ND)]
            # v8 pair-tiles: [t-in-chunk, chunk-parity plane, d] per t-pair
            vt8 = [pb.tile([128, 2, D], f8, tag=f"vt8_{tp}", bufs=1,
                           name=f"vt8_{tp}") for tp in range(NT // 2)]

            for t in range(NT):
                vps = {}
                for vb in range(2):
                    vps[vb] = psum()
                for dp in range(ND // 2):
                    for vb in range(2):
                        # stationary: xT8 pair [a2, t-chunk]; moving: wv8 pair
                        nc.tensor.matmul(
                            vps[vb][:],
                            xt8_a[:, 2 * dp:2 * dp + 2,
                                  t * 128:(t + 1) * 128],
                            wv_a[:, 2 * dp:2 * dp + 2,
                                 vb * 512:(vb + 1) * 512],
                            start=(dp == 0), stop=(dp == ND // 2 - 1),
                            perf_mode=DR)
                for vb in range(2):
                    nc.scalar.copy(
                        vt8[t // 2][:, t % 2, vb * 512:(vb + 1) * 512],
                        vps[vb][:])

            # ---- free phase-A inputs; right pool for attn + phase-C weights
            esA.close()
            pr = es.enter_context(tc.tile_pool(name="pAC", bufs=1,
                                               side="right"))
            wl_a = pr.tile([128, ND, D], b16, tag="wl", bufs=1)
            nc.sync.dma_start(wl_a[:], chunked(wlin_d, D))
            wf1_a = pr.tile([128, ND, D], b16, tag="wf1", bufs=1)
            nc.sync.dma_start(wf1_a[:], chunked(wff1_d, D))
            wf2_a = pr.tile([128, ND, D], b16, tag="wf2", bufs=1)
            nc.sync.dma_start(wf2_a[:], chunked(wff2_d, D))
            wlin_t = [wl_a[:, d] for d in range(ND)]
            wff1_t = [wf1_a[:, d] for d in range(ND)]
            wff2_t = [wf2_a[:, d] for d in range(ND)]

            attn = [pr.tile([128, 1024], b16, tag=f"at{d}", bufs=1,
                            name=f"at{d}") for d in range(ND)]

            def phase_b(parity):
                sb_slots = SB_SLOTS[parity]
                order = _mask_order(parity)
                qs = SB_GLOBAL[parity]
                # pass 0: yT[a,s] = sum_d wz[d,a] * x[s,d] for this core's
                # 1024 queries (two 512-wide column slices of xT)
                for m in range(ND):
                    yps = {h: psum() for h in (0, 1)}
                    for d in range(ND):
                        for h in (0, 1):
                            nc.tensor.matmul(
                                yps[h][:],
                                wz_t[d][:, m * 128:(m + 1) * 128],
                                xt[d][:, qs[h]:qs[h] + 512],
                                start=(d == 0), stop=(d == ND - 1))
                    for h in (0, 1):
                        nc.scalar.copy(
                            yt[m][:, h * 512:(h + 1) * 512], yps[h][:])

                # pass 1: scoresT -> exp -> mask -> den1, sb0-major so
                # den1(sb0) closes early and the DVE fp8-quantization chain
                # for sb0 hides under sb1's trailing score matmuls
                et = {}
                den1_ps = {sb: psum_den() for sb in (0, 1)}
                for ki, (sb, tcn) in enumerate(order):
                    scp = psum()
                    for a in range(ND):
                        nc.tensor.matmul(
                            scp[:],
                            xt[a][:, tcn * 128:(tcn + 1) * 128],
                            yt[a][:, sb * 512:(sb + 1) * 512],
                            start=(a == 0), stop=(a == ND - 1))
                    e = pb.tile([128, 512], b16, tag=f"et{ki}",
                                bufs=1, name=f"et{parity}_{sb}_{tcn}")
                    et[(sb, tcn)] = e
                    nc.scalar.activation(e[:], scp[:], AF.Exp)
                    mk = sp.tile([128, 512], b16, tag="mks", bufs=6,
                                 name=f"mk{parity}_{ki}")
                    nc.sync.dma_start(mk[:], masks_d.ap()[ki])
                    nc.vector.tensor_mul(e[:], e[:], mk[:])
                    slots = sb_slots[sb]
                    nc.tensor.matmul(
                        den1_ps[sb][:], ones_sq[:], e[:],
                        start=(tcn == slots[0]),
                        stop=(tcn == slots[-1]))

                # pass 1.5: rbs1, quantized+scaled weights et8 (sb0 first:
                # its den1 closed first under the sb0-major pass-1 order)
                et8 = {}
                rbs2 = {}
                for sb in (0, 1):
                    slots = sb_slots[sb]
                    r1 = sp.tile([128, 512], f32, tag="rbs1", bufs=2,
                                 name=f"rbs1_{parity}_{sb}")
                    nc.vector.reciprocal_approx_fast(r1[:], den1_ps[sb][:])
                    for tcn in slots:
                        tp = tcn // 2
                        if (sb, tp) not in et8:
                            et8[(sb, tp)] = pb.tile(
                                [128, 2, 512], f8, tag=f"et8_{sb}_{tp}",
                                bufs=1, name=f"et8_{parity}_{sb}_{tp}")
                        # et8 = (et * 64) * rbs1  (fp8, one DVE op)
                        nc.vector.scalar_tensor_tensor(
                            et8[(sb, tp)][:, tcn % 2],
                            et[(sb, tcn)][:], 64.0, r1[:],
                            ALU.mult, ALU.mult)
                    # den2 = sum_t 16*et8 (fp8 DR); rbs2 = 1/den2
                    den2_ps = psum()
                    tps = sorted({tcn // 2 for tcn in slots})
                    for k, tp in enumerate(tps):
                        nc.tensor.matmul(
                            den2_ps[:], ones8[:], et8[(sb, tp)][:],
                            start=(k == 0), stop=(k == len(tps) - 1),
                            perf_mode=DR)
                    r2 = sp.tile([128, 512], f32, tag="rbs2", bufs=2,
                                 name=f"rbs2_{parity}_{sb}")
                    nc.vector.reciprocal_approx_fast(r2[:], den2_ps[:])
                    rbs2[sb] = r2

                # pass 2: AV (fp8 DR) + renormalize -> attn (bf16)
                for sb in (0, 1):
                    slots = sb_slots[sb]
                    tps = sorted({tcn // 2 for tcn in slots})
                    for dc in range(ND):
                        avp = psum()
                        for k, tp in enumerate(tps):
                            nc.tensor.matmul(
                                avp[:],
                                vt8[tp][:, :, dc * 128:(dc + 1) * 128],
                                et8[(sb, tp)][:],
                                start=(k == 0), stop=(k == len(tps) - 1),
                                perf_mode=DR)
                        nc.vector.tensor_mul(
                            attn[dc][:, sb * 512:(sb + 1) * 512],
                            avp[:], rbs2[sb][:])

            par_regs = nc.alloc_registers("par_regs")
            nc.regs_load(par_regs, par_d.ap()[0:1, 0:1])
            par = nc.snap(par_regs, donate=True, min_val=0, max_val=1)
            with tc.If(par < 1) as cmp:
                phase_b(0)
            with cmp.Else():
                phase_b(1)

            # ---- free pB (zt/vt8/xt/et); left pool for phase-C tiles ----
            esB.close()
            esC = es.enter_context(ExitStack())
            pc = esC.enter_context(tc.tile_pool(name="pC", bufs=1))

            x2f = [pc.tile([128, 1024], f32, tag=f"x2f{d}", bufs=1,
                           name=f"x2f{d}") for d in range(ND)]
            x2b = [pc.tile([128, 1024], b16, tag=f"x2b{d}", bufs=1,
                           name=f"x2b{d}") for d in range(ND)]
            ht = [pc.tile([128, 1024], b16, tag=f"ht{d}", bufs=1,
                          name=f"ht{d}") for d in range(ND)]

            for oc in range(ND):
                for s2 in range(2):
                    cps = psum()
                    for d in range(ND):
                        nc.tensor.matmul(
                            cps[:],
                            wlin_t[d][:, oc * 128:(oc + 1) * 128],
                            attn[d][:, s2 * 512:(s2 + 1) * 512],
                            start=(d == 0), stop=(d == ND - 1))
                    xqt = sp.tile([128, 512], f16, tag="xqt", bufs=4,
                                  name=f"xqt{oc}_{s2}")
                    nc.sync.dma_start(
                        xqt[:],
                        xq_d.ap()[oc * 128:(oc + 1) * 128,
                                  s2 * 512:(s2 + 1) * 512])
                    cc = slice(s2 * 512, (s2 + 1) * 512)
                    nc.vector.tensor_add(x2f[oc][:, cc], cps[:], xqt[:])
                    nc.scalar.copy(x2b[oc][:, cc], x2f[oc][:, cc])

            for fc in range(ND):
                for s2 in range(2):
                    cps = psum()
                    for d in range(ND):
                        nc.tensor.matmul(
                            cps[:],
                            wff1_t[d][:, fc * 128:(fc + 1) * 128],
                            x2b[d][:, s2 * 512:(s2 + 1) * 512],
                            start=(d == 0), stop=(d == ND - 1))
                    cc = slice(s2 * 512, (s2 + 1) * 512)
                    nc.scalar.activation(ht[fc][:, cc], cps[:], AF.Relu,
                                         bias=bf1_t[:, fc:fc + 1])

            for oc in range(ND):
                for s2 in range(2):
                    cps = psum()
                    for f in range(ND):
                        nc.tensor.matmul(
                            cps[:],
                            wff2_t[f][:, oc * 128:(oc + 1) * 128],
                            ht[f][:, s2 * 512:(s2 + 1) * 512],
                            start=(f == 0), stop=(f == ND - 1))
                    cc = slice(s2 * 512, (s2 + 1) * 512)
                    ot = sp.tile([128, 512], f16, tag="ot", bufs=4,
                                 name=f"ot{oc}_{s2}")
                    nc.vector.tensor_add(ot[:], cps[:], x2f[oc][:, cc])
                    nc.sync.dma_start(
                        outT_d.ap()[oc * 128:(oc + 1) * 128, cc], ot[:])

    nc.compile()
    return nc


def _get_program():
    global _COMPILED
    if _COMPILED is None:
        _COMPILED = _build_program()
    return _COMPILED


def kernel(x, wqkv, w_lin, b_lin, w_ff1, b_ff1, w_ff2, b_ff2):
    from concourse.bass_utils import run_bass_kernel_spmd

    x = np.asarray(x, np.float32)
    wqkv = np.asarray(wqkv, np.float32)
    Wq = wqkv[:, :D].astype(np.float64)
    Wk = wqkv[:, D:2 * D].astype(np.float64)
    Wv = wqkv[:, 2 * D:]

    wz = ((Wq @ Wk.T) / 2.0).astype(BF16)       # lhsT layout [a, d]:
    # y = qx @ wz.T folds QK^T into the query side; device contracts dim0
    wv8 = (Wv * 16.0).astype(FP8)               # fp8, x16 for dynamic range
    wlin = np.asarray(w_lin, np.float32).astype(BF16)
    wff1 = np.asarray(w_ff1, np.float32).astype(BF16)
    wff2 = np.asarray(w_ff2, np.float32).astype(BF16)
    masks = {p: _build_masks(p) for p in (0, 1)}

    in_maps = []
    qcols_by_parity = {
        0: np.r_[0:512, 1536:2048],
        1: np.r_[512:1536],
    }
    b_lin = np.asarray(b_lin, np.float32)
    b_ff1 = np.asarray(b_ff1, np.float32)
    b_ff2 = np.asarray(b_ff2, np.float32)
    bf1 = np.ascontiguousarray(b_ff1.reshape(ND, 128))
    for c in range(NCORES):
        b, h = c // 2, c % 2
        xT32 = np.ascontiguousarray(x[b].T)               # [D, S] f32
        qcols = qcols_by_parity[h]
        xqT = np.ascontiguousarray(xT32[:, qcols]) + b_lin[:, None]
        in_maps.append({
            "xT": xT32.astype(BF16),
            "xT8": xT32.astype(FP8),
            "xq": xqT.astype(F16),                        # b_lin folded in
            "wz": wz,
            "wv": wv8,
            "wlin": wlin,
            "wff1": wff1,
            "wff2": wff2,
            "masks": masks[h],
            "bf1": bf1,
            "par": np.full((1, 1), h, np.uint32),
        })

    global _LAST_IN_MAPS
    _LAST_IN_MAPS = in_maps
    nc = _get_program()
    res = run_bass_kernel_spmd(nc, in_maps, core_ids=list(range(NCORES)))

    out = np.empty((B, S, D), np.float32)
    for c in range(NCORES):
        b, h = c // 2, c % 2
        ol = res.results[c]["outT"].astype(np.float32).T  # [1024 s, D]
        if h == 0:
            out[b, 0:512] = ol[:512]
            out[b, 1536:2048] = ol[512:]
        else:
            out[b, 512:1536] = ol
    out += b_ff2[None, None, :]
    return out


# revision 14
# speedup vs baseline: 1.4883x; 1.0055x over previous
"""Trainium2 Bass kernel for nn_MultiHeadAttention_48395691492077.

Reference (B=4, S=2048, D=1024, single head, anti-causal triu mask):
    qkv = x @ wqkv; q,k,v = split(qkv)
    scores = triu(q @ k^T / sqrt(B));  masked softmax over keys t >= s
    x2  = softmax(scores) @ v @ w_lin + b_lin + x
    out = relu(x2 @ w_ff1 + b_ff1) @ w_ff2 + b_ff2 + x2

Sharding: 8 cores = 4 batches x 2 query-halves. Each core computes the
full-batch key/value projections (duplicated within a pair) and attention
for its own 1024 queries. The program is identical on all cores (SPMD);
per-core differences (which queries, which mask pattern) are carried in
the input data.

Device algebra (everything transposed so no on-chip transposes are needed):
    yT[a,s] = sum_d wz[d,a] xT[d,s] over this core's 1024 queries only
        (wz = (Wq @ Wk^T)/2 host-precomputed, [a,d] layout; query-side
         fold = half the projection work of the key-side z = x @ wz form)
    scoresT[t,s] = sum_a xT[a,t] * yT[a,s]
    expT = exp(scoresT) * mask01  (bf16; no max-subtraction; |s|<=45)
    den1[s] = ones^T @ expT (PE);  rbs1 = 1/den1 (DVE)
    et8 = fp8(expT * rbs1 * 64)                    [normalized weights]
    den2 = (16*ones8)^T @ et8 (PE fp8 DoubleRow);  rbs2 = 1/den2
    v8 = fp8(xT8.T @ (16*wv8)) (fp8 DoubleRow)     [= 16*v]
    attnT = (v8.T @ et8) * rbs2 (DR + DVE)  -> renormalized softmax @ v
    x2T = w_lin.T @ attn^T + (xT + b_lin);  hT = relu(w_ff1.T @ x2T + b_ff1)
    outT = w_ff2.T @ hT + x2T               (+ b_ff2 added on host)

Precision split (validated on a CPU simulator of this exact algebra):
 - bf16 for all non-fp8 matmuls: fp16/fp8 operand streaming measures
   ~20% slower per matmul on this hardware (power/toggle throttle), and
   the fp8 AV noise dominates max-err anyway, so fp16 buys nothing.
 - v-projection + AV in fp8 e4m3 with DoubleRow (2x matmul throughput);
   the post-quantization renorm (den2) cancels the common-mode fp8 noise
   of the attention weights.  Simulated total rel-err ~1.7e-2 (< 2e-2).
 - FFN/w_lin in fp16 (fp8 there busts the error budget).
"""

import numpy as np
import ml_dtypes

B, S, D = 4, 2048, 1024
NCORES = 8
BF16 = ml_dtypes.bfloat16
FP8 = ml_dtypes.float8_e4m3fn
F16 = np.float16

NT = S // 128            # 16 t-chunks
ND = D // 128            # 8 chunks of 128 along any D-sized dim

# global query-column starts of (sb0, sb1) per parity
SB_GLOBAL = {0: (0, 1536), 1: (512, 1024)}
# t-chunks each (parity, s-block) actually needs (branch-specialized)
SB_SLOTS = {
    0: {0: list(range(0, NT)), 1: list(range(12, NT))},
    1: {0: list(range(4, NT)), 1: list(range(8, NT))},
}


_COMPILED = None
_LAST_IN_MAPS = None


def _mask_order(parity: int):
    return [(sb, tc) for sb in (0, 1) for tc in SB_SLOTS[parity][sb]]


def _build_masks(parity: int) -> np.ndarray:
    """[20, 128, 512] bf16 multiplicative masks, one per processed block."""
    order = _mask_order(parity)
    m = np.zeros((len(order), 128, 512), np.float32)
    ii = np.arange(128)[:, None]
    jj = np.arange(512)[None, :]
    for k, (sb, tc) in enumerate(order):
        s0 = SB_GLOBAL[parity][sb]
        m[k] = ((128 * tc + ii) >= (s0 + jj)).astype(np.float32)
    return m.astype(BF16)


def _build_program():
    from contextlib import ExitStack
    import concourse.bacc as bacc
    import concourse.mybir as mybir
    import concourse.tile as tile

    f32 = mybir.dt.float32
    b16 = mybir.dt.bfloat16
    f16 = mybir.dt.float16
    f8 = mybir.dt.float8e4
    AF = mybir.ActivationFunctionType
    ALU = mybir.AluOpType
    DR = mybir.MatmulPerfMode.DoubleRow

    nc = bacc.Bacc("TRN2", target_bir_lowering=False, debug=False,
                   num_devices=NCORES)

    xT_d = nc.dram_tensor("xT", [D, S], b16, kind="ExternalInput")
    xT8_d = nc.dram_tensor("xT8", [D, S], f8, kind="ExternalInput")
    xq_d = nc.dram_tensor("xq", [D, 1024], f16, kind="ExternalInput")
    wz_d = nc.dram_tensor("wz", [D, D], b16, kind="ExternalInput")
    wv_d = nc.dram_tensor("wv", [D, D], f8, kind="ExternalInput")
    wlin_d = nc.dram_tensor("wlin", [D, D], b16, kind="ExternalInput")
    wff1_d = nc.dram_tensor("wff1", [D, D], b16, kind="ExternalInput")
    wff2_d = nc.dram_tensor("wff2", [D, D], b16, kind="ExternalInput")
    masks_d = nc.dram_tensor("masks", [20, 128, 512], b16, kind="ExternalInput")
    par_d = nc.dram_tensor("par", [1, 1], mybir.dt.uint32, kind="ExternalInput")
    bf1_d = nc.dram_tensor("bf1", [ND, 128], f32, kind="ExternalInput")
    outT_d = nc.dram_tensor("outT", [D, 1024], f16, kind="ExternalOutput")

    with tile.TileContext(nc) as tc:
        es = ExitStack()
        with es:
            pp = es.enter_context(tc.tile_pool(name="persist", bufs=1))
            sp = es.enter_context(tc.tile_pool(name="stream", bufs=2))
            ps = es.enter_context(
                tc.tile_pool(name="ps", bufs=8, space="PSUM"))
            esB = es.enter_context(ExitStack())
            pb = esB.enter_context(tc.tile_pool(name="pB", bufs=1))
            esA = ExitStack()
            pa = esA.enter_context(tc.tile_pool(name="pA", bufs=1,
                                                side="right"))

            def psum():
                t = ps.tile([128, 512], f32, tag="mm", bufs=6, name="mmps")
                return t

            def psum_den():
                # dedicated banks: den1 accumulates across all of pass 1
                # while the "mm" tag keeps rotating
                return ps.tile([128, 512], f32, tag="den", bufs=2,
                               name="denps")

            # ---- constants ----
            ones_sq = pp.tile([128, 128], b16, tag="ones_sq", bufs=1)
            nc.vector.memset(ones_sq[:], 1.0)
            # fp8 "16s" for the renorm sum: folds the v-scale (16) into den2
            ones8 = pp.tile([128, 2, 128], f8, tag="ones8", bufs=1)
            nc.vector.memset(ones8[:], 16.0)
            # warm the PE HAM clock-gate while input DMAs are in flight
            wups = psum()
            for i in range(72):
                nc.tensor.matmul(wups[:, 0:128], ones_sq[:], ones_sq[:],
                                 start=(i == 0), stop=(i == 71))

            # parity register first: its 4-byte DMA must not queue behind
            # the 15MB of input loads (it gates the phase-B branch)
            par_regs = nc.alloc_registers("par_regs")
            nc.regs_load(par_regs, par_d.ap()[0:1, 0:1])
            par = nc.snap(par_regs, donate=True, min_val=0, max_val=1)

            # ---- input loads: one big DMA per tensor (issue-rate bound) ----
            def chunked(dram, cols):
                return dram.ap().rearrange("(c p) n -> p c n", p=128)

            # v-gemm inputs first: first xT8 column chunk, then both wv8
            # halves (vb=1 matmuls come 2nd in program order per t), then
            # the rest of xT8 in growing column chunks
            xt8_a = pa.tile([128, ND, S], f8, tag="xt8", bufs=1)
            wv_a = pa.tile([128, ND, D], f8, tag="wv", bufs=1)
            nc.sync.dma_start(
                xt8_a[:, :, 0:128],
                xT8_d.ap()[:, 0:128].rearrange("(c p) n -> p c n", p=128))
            for c0, c1 in ((0, 512), (512, 1024)):
                nc.sync.dma_start(wv_a[:, :, c0:c1],
                                  wv_d.ap()[:, c0:c1]
                                  .rearrange("(c p) n -> p c n", p=128))
            xt8_splits = [128, 256, 512, 1024, 1536, 2048]
            for c0, c1 in zip(xt8_splits, xt8_splits[1:]):
                nc.sync.dma_start(
                    xt8_a[:, :, c0:c1],
                    xT8_d.ap()[:, c0:c1]
                    .rearrange("(c p) n -> p c n", p=128))
            # z/scores inputs (fp16); xt lives in pB (used through phase B)
            xt_a = pb.tile([128, ND, S], b16, tag="xt", bufs=1)
            for c0, c1 in ((0, 1024), (1024, 2048)):
                nc.sync.dma_start(
                    xt_a[:, :, c0:c1],
                    xT_d.ap()[:, c0:c1]
                    .rearrange("(c p) n -> p c n", p=128))
            wz_a = pb.tile([128, ND, D], b16, tag="wz", bufs=1)
            nc.sync.dma_start(wz_a[:], chunked(wz_d, D))
            # b_ff1 laid out [128, ND]: bias column fc serves f-chunk fc
            bf1_t = pp.tile([128, ND], f32, tag="bf1", bufs=1)
            nc.sync.dma_start(bf1_t[:], bf1_d.ap().rearrange("c p -> p c"))
            wv_t = [wv_a[:, d] for d in range(ND)]
            xt = [xt_a[:, d] for d in range(ND)]
            wz_t = [wz_a[:, d] for d in range(ND)]

            # ---- phase A: v8 [t,d] (fp8 DR); yT is computed in the
            # parity branch (query-side fold: half the work of key-side z)
            yt = [pb.tile([128, 1024], b16, tag=f"yt{m}", bufs=1,
                          name=f"yt{m}") for m in range(ND)]
            # v8 pair-tiles: [t-in-chunk, chunk-parity plane, d] per t-pair
            vt8 = [pb.tile([128, 2, D], f8, tag=f"vt8_{tp}", bufs=1,
                           name=f"vt8_{tp}") for tp in range(NT // 2)]

            for t in range(NT):
                vps = {}
                for vb in range(2):
                    vps[vb] = psum()
                for dp in range(ND // 2):
                    for vb in range(2):
                        # stationary: xT8 pair [a2, t-chunk]; moving: wv8 pair
                        nc.tensor.matmul(
                            vps[vb][:],
                            xt8_a[:, 2 * dp:2 * dp + 2,
                                  t * 128:(t + 1) * 128],
                            wv_a[:, 2 * dp:2 * dp + 2,
                                 vb * 512:(vb + 1) * 512],
                            start=(dp == 0), stop=(dp == ND // 2 - 1),
                            perf_mode=DR)
                for vb in range(2):
                    nc.scalar.copy(
                        vt8[t // 2][:, t % 2, vb * 512:(vb + 1) * 512],
                        vps[vb][:])

            # ---- free phase-A inputs; right pool for attn + phase-C weights
            esA.close()
            pr = es.enter_context(tc.tile_pool(name="pAC", bufs=1,
                                               side="right"))
            wl_a = pr.tile([128, ND, D], b16, tag="wl", bufs=1)
            nc.sync.dma_start(wl_a[:], chunked(wlin_d, D))
            wf1_a = pr.tile([128, ND, D], b16, tag="wf1", bufs=1)
            nc.sync.dma_start(wf1_a[:], chunked(wff1_d, D))
            wf2_a = pr.tile([128, ND, D], b16, tag="wf2", bufs=1)
            nc.sync.dma_start(wf2_a[:], chunked(wff2_d, D))
            wlin_t = [wl_a[:, d] for d in range(ND)]
            wff1_t = [wf1_a[:, d] for d in range(ND)]
            wff2_t = [wf2_a[:, d] for d in range(ND)]

            attn = [pr.tile([128, 1024], b16, tag=f"at{d}", bufs=1,
                            name=f"at{d}") for d in range(ND)]

            def phase_b(parity):
                sb_slots = SB_SLOTS[parity]
                order = _mask_order(parity)
                qs = SB_GLOBAL[parity]
                # pass 0: yT[a,s] = sum_d wz[d,a] * x[s,d] for this core's
                # 1024 queries (two 512-wide column slices of xT)
                for m in range(ND):
                    yps = {h: psum() for h in (0, 1)}
                    for d in range(ND):
                        for h in (0, 1):
                            nc.tensor.matmul(
                                yps[h][:],
                                wz_t[d][:, m * 128:(m + 1) * 128],
                                xt[d][:, qs[h]:qs[h] + 512],
                                start=(d == 0), stop=(d == ND - 1))
                    for h in (0, 1):
                        nc.scalar.copy(
                            yt[m][:, h * 512:(h + 1) * 512], yps[h][:])

                # pass 1: scoresT -> exp -> mask -> den1, sb0-major so
                # den1(sb0) closes early and the DVE fp8-quantization chain
                # for sb0 hides under sb1's trailing score matmuls
                et = {}
                den1_ps = {sb: psum_den() for sb in (0, 1)}
                for ki, (sb, tcn) in enumerate(order):
                    scp = psum()
                    for a in range(ND):
                        nc.tensor.matmul(
                            scp[:],
                            xt[a][:, tcn * 128:(tcn + 1) * 128],
                            yt[a][:, sb * 512:(sb + 1) * 512],
                            start=(a == 0), stop=(a == ND - 1))
                    e = pb.tile([128, 512], b16, tag=f"et{ki}",
                                bufs=1, name=f"et{parity}_{sb}_{tcn}")
                    et[(sb, tcn)] = e
                    nc.scalar.activation(e[:], scp[:], AF.Exp)
                    mk = sp.tile([128, 512], b16, tag="mks", bufs=6,
                                 name=f"mk{parity}_{ki}")
                    nc.sync.dma_start(mk[:], masks_d.ap()[ki])
                    nc.vector.tensor_mul(e[:], e[:], mk[:])
                    slots = sb_slots[sb]
                    nc.tensor.matmul(
                        den1_ps[sb][:], ones_sq[:], e[:],
                        start=(tcn == slots[0]),
                        stop=(tcn == slots[-1]))

                # pass 1.5 + 2: both rbs1 upfront, then per sb the et8
                # quantization interleaves with den2 and the first AV chain
                # pair-by-pair (trickle-feeds the PE during the DVE chain);
                # remaining AV chains run once all et8 pairs exist
                rbs1 = {}
                for sb in (0, 1):
                    r1 = sp.tile([128, 512], f32, tag="rbs1", bufs=2,
                                 name=f"rbs1_{parity}_{sb}")
                    nc.vector.reciprocal_approx_fast(r1[:], den1_ps[sb][:])
                    rbs1[sb] = r1
                et8 = {}
                rbs2 = {}
                for sb in (0, 1):
                    slots = sb_slots[sb]
                    tps = sorted({tcn // 2 for tcn in slots})
                    den2_ps = psum_den()
                    av0_ps = psum_den()
                    for k, tp in enumerate(tps):
                        et8[(sb, tp)] = pb.tile(
                            [128, 2, 512], f8, tag=f"et8_{sb}_{tp}",
                            bufs=1, name=f"et8_{parity}_{sb}_{tp}")
                        for j in (0, 1):
                            nc.vector.scalar_tensor_tensor(
                                et8[(sb, tp)][:, j],
                                et[(sb, 2 * tp + j)][:], 64.0, rbs1[sb][:],
                                ALU.mult, ALU.mult)
                        st, sp_ = (k == 0), (k == len(tps) - 1)
                        nc.tensor.matmul(
                            den2_ps[:], ones8[:], et8[(sb, tp)][:],
                            start=st, stop=sp_, perf_mode=DR)
                        nc.tensor.matmul(
                            av0_ps[:], vt8[tp][:, :, 0:128],
                            et8[(sb, tp)][:], start=st, stop=sp_,
                            perf_mode=DR)
                    r2 = sp.tile([128, 512], f32, tag="rbs2", bufs=2,
                                 name=f"rbs2_{parity}_{sb}")
                    nc.vector.reciprocal_approx_fast(r2[:], den2_ps[:])
                    rbs2[sb] = r2
                    nc.vector.tensor_mul(
                        attn[0][:, sb * 512:(sb + 1) * 512],
                        av0_ps[:], r2[:])
                    for dc in range(1, ND):
                        avp = psum()
                        for k, tp in enumerate(tps):
                            nc.tensor.matmul(
                                avp[:],
                                vt8[tp][:, :, dc * 128:(dc + 1) * 128],
                                et8[(sb, tp)][:],
                                start=(k == 0), stop=(k == len(tps) - 1),
                                perf_mode=DR)
                        nc.vector.tensor_mul(
                            attn[dc][:, sb * 512:(sb + 1) * 512],
                            avp[:], rbs2[sb][:])

            with tc.If(par < 1) as cmp:
                phase_b(0)
            with cmp.Else():
                phase_b(1)

            # ---- free pB (zt/vt8/xt/et); left pool for phase-C tiles ----
            esB.close()
            esC = es.enter_context(ExitStack())
            pc = esC.enter_context(tc.tile_pool(name="pC", bufs=1))

            x2f = [pc.tile([128, 1024], f32, tag=f"x2f{d}", bufs=1,
                           name=f"x2f{d}") for d in range(ND)]
            x2b = [pc.tile([128, 1024], b16, tag=f"x2b{d}", bufs=1,
                           name=f"x2b{d}") for d in range(ND)]
            ht = [pc.tile([128, 1024], b16, tag=f"ht{d}", bufs=1,
                          name=f"ht{d}") for d in range(ND)]

            for s2 in range(2):
                for oc in range(ND):
                    cps = psum()
                    for d in range(ND):
                        nc.tensor.matmul(
                            cps[:],
                            wlin_t[d][:, oc * 128:(oc + 1) * 128],
                            attn[d][:, s2 * 512:(s2 + 1) * 512],
                            start=(d == 0), stop=(d == ND - 1))
                    xqt = sp.tile([128, 512], f16, tag="xqt", bufs=4,
                                  name=f"xqt{oc}_{s2}")
                    nc.sync.dma_start(
                        xqt[:],
                        xq_d.ap()[oc * 128:(oc + 1) * 128,
                                  s2 * 512:(s2 + 1) * 512])
                    cc = slice(s2 * 512, (s2 + 1) * 512)
                    nc.vector.tensor_add(x2f[oc][:, cc], cps[:], xqt[:])
                    nc.scalar.copy(x2b[oc][:, cc], x2f[oc][:, cc])

            for s2 in range(2):
                for fc in range(ND):
                    cps = psum()
                    for d in range(ND):
                        nc.tensor.matmul(
                            cps[:],
                            wff1_t[d][:, fc * 128:(fc + 1) * 128],
                            x2b[d][:, s2 * 512:(s2 + 1) * 512],
                            start=(d == 0), stop=(d == ND - 1))
                    cc = slice(s2 * 512, (s2 + 1) * 512)
                    nc.scalar.activation(ht[fc][:, cc], cps[:], AF.Relu,
                                         bias=bf1_t[:, fc:fc + 1])

            for s2 in range(2):
                for oc in range(ND):
                    cps = psum()
                    for f in range(ND):
                        nc.tensor.matmul(
                            cps[:],
                            wff2_t[f][:, oc * 128:(oc + 1) * 128],
                            ht[f][:, s2 * 512:(s2 + 1) * 512],
                            start=(f == 0), stop=(f == ND - 1))
                    cc = slice(s2 * 512, (s2 + 1) * 512)
                    ot = sp.tile([128, 512], f16, tag="ot", bufs=4,
                                 name=f"ot{oc}_{s2}")
                    nc.vector.tensor_add(ot[:], cps[:], x2f[oc][:, cc])
                    nc.sync.dma_start(
                        outT_d.ap()[oc * 128:(oc + 1) * 128, cc], ot[:])

    nc.compile()
    return nc


def _get_program():
    global _COMPILED
    if _COMPILED is None:
        _COMPILED = _build_program()
    return _COMPILED


def kernel(x, wqkv, w_lin, b_lin, w_ff1, b_ff1, w_ff2, b_ff2):
    from concourse.bass_utils import run_bass_kernel_spmd

    x = np.asarray(x, np.float32)
    wqkv = np.asarray(wqkv, np.float32)
    Wq = wqkv[:, :D].astype(np.float64)
    Wk = wqkv[:, D:2 * D].astype(np.float64)
    Wv = wqkv[:, 2 * D:]

    wz = ((Wq @ Wk.T) / 2.0).astype(BF16)       # lhsT layout [a, d]:
    # y = qx @ wz.T folds QK^T into the query side; device contracts dim0
    wv8 = (Wv * 16.0).astype(FP8)               # fp8, x16 for dynamic range
    wlin = np.asarray(w_lin, np.float32).astype(BF16)
    wff1 = np.asarray(w_ff1, np.float32).astype(BF16)
    wff2 = np.asarray(w_ff2, np.float32).astype(BF16)
    masks = {p: _build_masks(p) for p in (0, 1)}

    in_maps = []
    qcols_by_parity = {
        0: np.r_[0:512, 1536:2048],
        1: np.r_[512:1536],
    }
    b_lin = np.asarray(b_lin, np.float32)
    b_ff1 = np.asarray(b_ff1, np.float32)
    b_ff2 = np.asarray(b_ff2, np.float32)
    bf1 = np.ascontiguousarray(b_ff1.reshape(ND, 128))
    for c in range(NCORES):
        b, h = c // 2, c % 2
        xT32 = np.ascontiguousarray(x[b].T)               # [D, S] f32
        qcols = qcols_by_parity[h]
        xqT = np.ascontiguousarray(xT32[:, qcols]) + b_lin[:, None]
        in_maps.append({
            "xT": xT32.astype(BF16),
            "xT8": xT32.astype(FP8),
            "xq": xqT.astype(F16),                        # b_lin folded in
            "wz": wz,
            "wv": wv8,
            "wlin": wlin,
            "wff1": wff1,
            "wff2": wff2,
            "masks": masks[h],
            "bf1": bf1,
            "par": np.full((1, 1), h, np.uint32),
        })

    global _LAST_IN_MAPS
    _LAST_IN_MAPS = in_maps
    nc = _get_program()
    res = run_bass_kernel_spmd(nc, in_maps, core_ids=list(range(NCORES)))

    out = np.empty((B, S, D), np.float32)
    for c in range(NCORES):
        b, h = c // 2, c % 2
        ol = res.results[c]["outT"].astype(np.float32).T  # [1024 s, D]
        if h == 0:
            out[b, 0:512] = ol[:512]
            out[b, 1536:2048] = ol[512:]
        else:
            out[b, 512:1536] = ol
    out += b_ff2[None, None, :]
    return out


# revision 15
# speedup vs baseline: 1.5487x; 1.0406x over previous
"""Trainium2 Bass kernel for nn_MultiHeadAttention_48395691492077.

Reference (B=4, S=2048, D=1024, single head, anti-causal triu mask):
    qkv = x @ wqkv; q,k,v = split(qkv)
    scores = triu(q @ k^T / sqrt(B));  masked softmax over keys t >= s
    x2  = softmax(scores) @ v @ w_lin + b_lin + x
    out = relu(x2 @ w_ff1 + b_ff1) @ w_ff2 + b_ff2 + x2

Sharding: 8 cores = 4 batches x 2 query-halves. Each core computes the
full-batch key/value projections (duplicated within a pair) and attention
for its own 1024 queries. The program is identical on all cores (SPMD);
per-core differences (which queries, which mask pattern) are carried in
the input data.

Device algebra (everything transposed so no on-chip transposes are needed):
    yT[a,s] = sum_d wz[d,a] xT[d,s] over this core's 1024 queries only
        (wz = (Wq @ Wk^T)/2 host-precomputed, [a,d] layout; query-side
         fold = half the projection work of the key-side z = x @ wz form)
    scoresT[t,s] = sum_a xT[a,t] * yT[a,s]
    expT = exp(scoresT) * mask01  (bf16; no max-subtraction; |s|<=45)
    den1[s] = ones^T @ expT (PE);  rbs1 = 1/den1 (DVE)
    et8 = fp8(expT * rbs1 * 64)                    [normalized weights]
    den2 = (16*ones8)^T @ et8 (PE fp8 DoubleRow);  rbs2 = 1/den2
    v8 = fp8(xT8.T @ (16*wv8)) (fp8 DoubleRow)     [= 16*v]
    attnT = (v8.T @ et8) * rbs2 (DR + DVE)  -> renormalized softmax @ v
    x2T = w_lin.T @ attn^T + (xT + b_lin);  hT = relu(w_ff1.T @ x2T + b_ff1)
    outT = w_ff2.T @ hT + x2T               (+ b_ff2 added on host)

Precision split (validated on a CPU simulator of this exact algebra):
 - bf16 for all non-fp8 matmuls: fp16/fp8 operand streaming measures
   ~20% slower per matmul on this hardware (power/toggle throttle), and
   the fp8 AV noise dominates max-err anyway, so fp16 buys nothing.
 - v-projection + AV in fp8 e4m3 with DoubleRow (2x matmul throughput);
   the post-quantization renorm (den2) cancels the common-mode fp8 noise
   of the attention weights.  Simulated total rel-err ~1.7e-2 (< 2e-2).
 - FFN/w_lin in fp16 (fp8 there busts the error budget).
"""

import numpy as np
import ml_dtypes

B, S, D = 4, 2048, 1024
NCORES = 8
BF16 = ml_dtypes.bfloat16
FP8 = ml_dtypes.float8_e4m3fn
F16 = np.float16

NT = S // 128            # 16 t-chunks
ND = D // 128            # 8 chunks of 128 along any D-sized dim

# global query-column starts of (sb0, sb1) per parity
SB_GLOBAL = {0: (0, 1536), 1: (512, 1024)}
# t-chunks each (parity, s-block) actually needs (branch-specialized)
SB_SLOTS = {
    0: {0: list(range(0, NT)), 1: list(range(12, NT))},
    1: {0: list(range(4, NT)), 1: list(range(8, NT))},
}


_COMPILED = None
_LAST_IN_MAPS = None


def _mask_order(parity: int):
    return [(sb, tc) for sb in (0, 1) for tc in SB_SLOTS[parity][sb]]


def _build_masks(parity: int) -> np.ndarray:
    """[20, 128, 512] bf16 multiplicative masks, one per processed block."""
    order = _mask_order(parity)
    m = np.zeros((len(order), 128, 512), np.float32)
    ii = np.arange(128)[:, None]
    jj = np.arange(512)[None, :]
    for k, (sb, tc) in enumerate(order):
        s0 = SB_GLOBAL[parity][sb]
        m[k] = ((128 * tc + ii) >= (s0 + jj)).astype(np.float32)
    return m.astype(BF16)


def _build_program():
    from contextlib import ExitStack
    import concourse.bacc as bacc
    import concourse.mybir as mybir
    import concourse.tile as tile

    f32 = mybir.dt.float32
    b16 = mybir.dt.bfloat16
    f16 = mybir.dt.float16
    f8 = mybir.dt.float8e4
    AF = mybir.ActivationFunctionType
    ALU = mybir.AluOpType
    DR = mybir.MatmulPerfMode.DoubleRow

    nc = bacc.Bacc("TRN2", target_bir_lowering=False, debug=False,
                   num_devices=NCORES)

    xT_d = nc.dram_tensor("xT", [D, S], b16, kind="ExternalInput")
    xT8_d = nc.dram_tensor("xT8", [D, S], f8, kind="ExternalInput")
    xq_d = nc.dram_tensor("xq", [D, 1024], f16, kind="ExternalInput")
    wz_d = nc.dram_tensor("wz", [D, D], b16, kind="ExternalInput")
    wv_d = nc.dram_tensor("wv", [D, D], f8, kind="ExternalInput")
    wlin_d = nc.dram_tensor("wlin", [D, D], b16, kind="ExternalInput")
    wff1_d = nc.dram_tensor("wff1", [D, D], b16, kind="ExternalInput")
    wff2_d = nc.dram_tensor("wff2", [D, D], b16, kind="ExternalInput")
    masks_d = nc.dram_tensor("masks", [20, 128, 512], b16, kind="ExternalInput")
    par_d = nc.dram_tensor("par", [1, 1], mybir.dt.uint32, kind="ExternalInput")
    bf1_d = nc.dram_tensor("bf1", [ND, 128], f32, kind="ExternalInput")
    outT_d = nc.dram_tensor("outT", [D, 1024], f16, kind="ExternalOutput")

    with tile.TileContext(nc) as tc:
        es = ExitStack()
        with es:
            pp = es.enter_context(tc.tile_pool(name="persist", bufs=1))
            sp = es.enter_context(tc.tile_pool(name="stream", bufs=2))
            ps = es.enter_context(
                tc.tile_pool(name="ps", bufs=8, space="PSUM"))
            esB = es.enter_context(ExitStack())
            pb = esB.enter_context(tc.tile_pool(name="pB", bufs=1))
            esA = ExitStack()
            pa = esA.enter_context(tc.tile_pool(name="pA", bufs=1,
                                                side="right"))

            def psum():
                t = ps.tile([128, 512], f32, tag="mm", bufs=6, name="mmps")
                return t

            def psum_den():
                # dedicated banks: den1 accumulates across all of pass 1
                # while the "mm" tag keeps rotating
                return ps.tile([128, 512], f32, tag="den", bufs=2,
                               name="denps")

            # ---- constants ----
            ones_sq = pp.tile([128, 128], b16, tag="ones_sq", bufs=1)
            nc.vector.memset(ones_sq[:], 1.0)
            # fp8 "16s" for the renorm sum: folds the v-scale (16) into den2
            ones8 = pp.tile([128, 2, 128], f8, tag="ones8", bufs=1)
            nc.vector.memset(ones8[:], 16.0)
            # warm the PE HAM clock-gate while input DMAs are in flight
            wups = psum()
            for i in range(88):
                nc.tensor.matmul(wups[:, 0:128], ones_sq[:], ones_sq[:],
                                 start=(i == 0), stop=(i == 87))

            # parity register first: its 4-byte DMA must not queue behind
            # the 15MB of input loads (it gates the phase-B branch)
            par_regs = nc.alloc_registers("par_regs")
            nc.regs_load(par_regs, par_d.ap()[0:1, 0:1])
            par = nc.snap(par_regs, donate=True, min_val=0, max_val=1)

            # ---- input loads: one big DMA per tensor (issue-rate bound) ----
            def chunked(dram, cols):
                return dram.ap().rearrange("(c p) n -> p c n", p=128)

            # v-gemm inputs first: first xT8 column chunk, then both wv8
            # halves (vb=1 matmuls come 2nd in program order per t), then
            # the rest of xT8 in growing column chunks
            xt8_a = pa.tile([128, ND, S], f8, tag="xt8", bufs=1)
            wv_a = pa.tile([128, ND, D], f8, tag="wv", bufs=1)
            nc.sync.dma_start(
                xt8_a[:, :, 0:128],
                xT8_d.ap()[:, 0:128].rearrange("(c p) n -> p c n", p=128))
            for c0, c1 in ((0, 512), (512, 1024)):
                nc.sync.dma_start(wv_a[:, :, c0:c1],
                                  wv_d.ap()[:, c0:c1]
                                  .rearrange("(c p) n -> p c n", p=128))
            xt8_splits = [128, 256, 512, 1024, 1536, 2048]
            for c0, c1 in zip(xt8_splits, xt8_splits[1:]):
                nc.sync.dma_start(
                    xt8_a[:, :, c0:c1],
                    xT8_d.ap()[:, c0:c1]
                    .rearrange("(c p) n -> p c n", p=128))
            # z/scores inputs (fp16); xt lives in pB (used through phase B)
            xt_a = pb.tile([128, ND, S], b16, tag="xt", bufs=1)
            for c0, c1 in ((0, 1024), (1024, 2048)):
                nc.sync.dma_start(
                    xt_a[:, :, c0:c1],
                    xT_d.ap()[:, c0:c1]
                    .rearrange("(c p) n -> p c n", p=128))
            wz_a = pb.tile([128, ND, D], b16, tag="wz", bufs=1)
            nc.sync.dma_start(wz_a[:], chunked(wz_d, D))
            # b_ff1 laid out [128, ND]: bias column fc serves f-chunk fc
            bf1_t = pp.tile([128, ND], f32, tag="bf1", bufs=1)
            nc.sync.dma_start(bf1_t[:], bf1_d.ap().rearrange("c p -> p c"))
            wv_t = [wv_a[:, d] for d in range(ND)]
            xt = [xt_a[:, d] for d in range(ND)]
            wz_t = [wz_a[:, d] for d in range(ND)]

            # ---- phase A: v8 [t,d] (fp8 DR); yT is computed in the
            # parity branch (query-side fold: half the work of key-side z)
            yt = [pb.tile([128, 1024], b16, tag=f"yt{m}", bufs=1,
                          name=f"yt{m}") for m in range(ND)]
            # v8 pair-tiles: [t-in-chunk, chunk-parity plane, d] per t-pair
            vt8 = [pb.tile([128, 2, D], f8, tag=f"vt8_{tp}", bufs=1,
                           name=f"vt8_{tp}") for tp in range(NT // 2)]

            for t in range(NT):
                vps = {}
                for vb in range(2):
                    vps[vb] = psum()
                for dp in range(ND // 2):
                    for vb in range(2):
                        # stationary: xT8 pair [a2, t-chunk]; moving: wv8 pair
                        nc.tensor.matmul(
                            vps[vb][:],
                            xt8_a[:, 2 * dp:2 * dp + 2,
                                  t * 128:(t + 1) * 128],
                            wv_a[:, 2 * dp:2 * dp + 2,
                                 vb * 512:(vb + 1) * 512],
                            start=(dp == 0), stop=(dp == ND // 2 - 1),
                            perf_mode=DR)
                for vb in range(2):
                    nc.scalar.copy(
                        vt8[t // 2][:, t % 2, vb * 512:(vb + 1) * 512],
                        vps[vb][:])

            # ---- free phase-A inputs; right pool for attn + phase-C weights
            esA.close()
            pr = es.enter_context(tc.tile_pool(name="pAC", bufs=1,
                                               side="right"))
            wl_a = pr.tile([128, ND, D], b16, tag="wl", bufs=1)
            nc.sync.dma_start(wl_a[:], chunked(wlin_d, D))
            wf1_a = pr.tile([128, ND, D], b16, tag="wf1", bufs=1)
            nc.sync.dma_start(wf1_a[:], chunked(wff1_d, D))
            wf2_a = pr.tile([128, ND, D], b16, tag="wf2", bufs=1)
            nc.sync.dma_start(wf2_a[:], chunked(wff2_d, D))
            wlin_t = [wl_a[:, d] for d in range(ND)]
            wff1_t = [wf1_a[:, d] for d in range(ND)]
            wff2_t = [wf2_a[:, d] for d in range(ND)]

            attn = [pr.tile([128, 1024], b16, tag=f"at{d}", bufs=1,
                            name=f"at{d}") for d in range(ND)]

            def phase_b(parity):
                sb_slots = SB_SLOTS[parity]
                order = _mask_order(parity)
                qs = SB_GLOBAL[parity]
                # pass 0: yT[a,s] = sum_d wz[d,a] * x[s,d] for this core's
                # 1024 queries (two 512-wide column slices of xT)
                for m in range(ND):
                    yps = {h: psum() for h in (0, 1)}
                    for d in range(ND):
                        for h in (0, 1):
                            nc.tensor.matmul(
                                yps[h][:],
                                wz_t[d][:, m * 128:(m + 1) * 128],
                                xt[d][:, qs[h]:qs[h] + 512],
                                start=(d == 0), stop=(d == ND - 1))
                    for h in (0, 1):
                        nc.scalar.copy(
                            yt[m][:, h * 512:(h + 1) * 512], yps[h][:])

                # pass 1: scoresT -> exp -> mask -> den1, sb0-major so
                # den1(sb0) closes early and the DVE fp8-quantization chain
                # for sb0 hides under sb1's trailing score matmuls
                et = {}
                den1_ps = {sb: psum_den() for sb in (0, 1)}
                for ki, (sb, tcn) in enumerate(order):
                    # columns beyond the triangle are fully masked: trim the
                    # score matmuls to the valid width on diagonal blocks
                    nv = min(512, 128 * (tcn + 1) - qs[sb])
                    scp = psum()
                    for a in range(ND):
                        nc.tensor.matmul(
                            scp[:, 0:nv],
                            xt[a][:, tcn * 128:(tcn + 1) * 128],
                            yt[a][:, sb * 512:sb * 512 + nv],
                            start=(a == 0), stop=(a == ND - 1))
                    e = pb.tile([128, 512], b16, tag=f"et{ki}",
                                bufs=1, name=f"et{parity}_{sb}_{tcn}")
                    et[(sb, tcn)] = e
                    nc.scalar.activation(e[:, 0:nv], scp[:, 0:nv], AF.Exp)
                    if nv < 512:
                        nc.vector.memset(e[:, nv:512], 0.0)
                    mk = sp.tile([128, 512], b16, tag="mks", bufs=6,
                                 name=f"mk{parity}_{ki}")
                    nc.sync.dma_start(mk[:, 0:nv], masks_d.ap()[ki][:, 0:nv])
                    nc.vector.tensor_mul(e[:, 0:nv], e[:, 0:nv], mk[:, 0:nv])
                    slots = sb_slots[sb]
                    nc.tensor.matmul(
                        den1_ps[sb][:], ones_sq[:], e[:],
                        start=(tcn == slots[0]),
                        stop=(tcn == slots[-1]))

                # pass 1.5 + 2: both rbs1 upfront, then per sb the et8
                # quantization interleaves with den2 and the first AV chain
                # pair-by-pair (trickle-feeds the PE during the DVE chain);
                # remaining AV chains run once all et8 pairs exist
                rbs1 = {}
                for sb in (0, 1):
                    r1 = sp.tile([128, 512], f32, tag="rbs1", bufs=2,
                                 name=f"rbs1_{parity}_{sb}")
                    nc.vector.reciprocal_approx_fast(r1[:], den1_ps[sb][:])
                    rbs1[sb] = r1
                et8 = {}
                rbs2 = {}
                for sb in (0, 1):
                    slots = sb_slots[sb]
                    tps = sorted({tcn // 2 for tcn in slots})
                    den2_ps = psum_den()
                    av0_ps = psum_den()
                    for k, tp in enumerate(tps):
                        et8[(sb, tp)] = pb.tile(
                            [128, 2, 512], f8, tag=f"et8_{sb}_{tp}",
                            bufs=1, name=f"et8_{parity}_{sb}_{tp}")
                        for j in (0, 1):
                            nc.vector.scalar_tensor_tensor(
                                et8[(sb, tp)][:, j],
                                et[(sb, 2 * tp + j)][:], 64.0, rbs1[sb][:],
                                ALU.mult, ALU.mult)
                        st, sp_ = (k == 0), (k == len(tps) - 1)
                        nc.tensor.matmul(
                            den2_ps[:], ones8[:], et8[(sb, tp)][:],
                            start=st, stop=sp_, perf_mode=DR)
                        nc.tensor.matmul(
                            av0_ps[:], vt8[tp][:, :, 0:128],
                            et8[(sb, tp)][:], start=st, stop=sp_,
                            perf_mode=DR)
                    r2 = sp.tile([128, 512], f32, tag="rbs2", bufs=2,
                                 name=f"rbs2_{parity}_{sb}")
                    nc.vector.reciprocal_approx_fast(r2[:], den2_ps[:])
                    rbs2[sb] = r2
                    nc.vector.tensor_mul(
                        attn[0][:, sb * 512:(sb + 1) * 512],
                        av0_ps[:], r2[:])
                    for dc in range(1, ND):
                        avp = psum()
                        for k, tp in enumerate(tps):
                            nc.tensor.matmul(
                                avp[:],
                                vt8[tp][:, :, dc * 128:(dc + 1) * 128],
                                et8[(sb, tp)][:],
                                start=(k == 0), stop=(k == len(tps) - 1),
                                perf_mode=DR)
                        nc.vector.tensor_mul(
                            attn[dc][:, sb * 512:(sb + 1) * 512],
                            avp[:], rbs2[sb][:])

            with tc.If(par < 1) as cmp:
                phase_b(0)
            with cmp.Else():
                phase_b(1)

            # ---- free pB (zt/vt8/xt/et); left pool for phase-C tiles ----
            esB.close()
            esC = es.enter_context(ExitStack())
            pc = esC.enter_context(tc.tile_pool(name="pC", bufs=1))

            x2f = [pc.tile([128, 1024], f32, tag=f"x2f{d}", bufs=1,
                           name=f"x2f{d}") for d in range(ND)]
            x2b = [pc.tile([128, 1024], b16, tag=f"x2b{d}", bufs=1,
                           name=f"x2b{d}") for d in range(ND)]
            ht = [pc.tile([128, 1024], b16, tag=f"ht{d}", bufs=1,
                          name=f"ht{d}") for d in range(ND)]

            for s2 in range(2):
                for oc in range(ND):
                    cps = psum()
                    for d in range(ND):
                        nc.tensor.matmul(
                            cps[:],
                            wlin_t[d][:, oc * 128:(oc + 1) * 128],
                            attn[d][:, s2 * 512:(s2 + 1) * 512],
                            start=(d == 0), stop=(d == ND - 1))
                    xqt = sp.tile([128, 512], f16, tag="xqt", bufs=4,
                                  name=f"xqt{oc}_{s2}")
                    nc.sync.dma_start(
                        xqt[:],
                        xq_d.ap()[oc * 128:(oc + 1) * 128,
                                  s2 * 512:(s2 + 1) * 512])
                    cc = slice(s2 * 512, (s2 + 1) * 512)
                    nc.vector.tensor_add(x2f[oc][:, cc], cps[:], xqt[:])
                    nc.scalar.copy(x2b[oc][:, cc], x2f[oc][:, cc])

            for s2 in range(2):
                for fc in range(ND):
                    cps = psum()
                    for d in range(ND):
                        nc.tensor.matmul(
                            cps[:],
                            wff1_t[d][:, fc * 128:(fc + 1) * 128],
                            x2b[d][:, s2 * 512:(s2 + 1) * 512],
                            start=(d == 0), stop=(d == ND - 1))
                    cc = slice(s2 * 512, (s2 + 1) * 512)
                    nc.scalar.activation(ht[fc][:, cc], cps[:], AF.Relu,
                                         bias=bf1_t[:, fc:fc + 1])

            for s2 in range(2):
                for oc in range(ND):
                    cps = psum()
                    for f in range(ND):
                        nc.tensor.matmul(
                            cps[:],
                            wff2_t[f][:, oc * 128:(oc + 1) * 128],
                            ht[f][:, s2 * 512:(s2 + 1) * 512],
                            start=(f == 0), stop=(f == ND - 1))
                    cc = slice(s2 * 512, (s2 + 1) * 512)
                    ot = sp.tile([128, 512], f16, tag="ot", bufs=4,
                                 name=f"ot{oc}_{s2}")
                    nc.vector.tensor_add(ot[:], cps[:], x2f[oc][:, cc])
                    nc.sync.dma_start(
                        outT_d.ap()[oc * 128:(oc + 1) * 128, cc], ot[:])

    nc.compile()
    return nc


def _get_program():
    global _COMPILED
    if _COMPILED is None:
        _COMPILED = _build_program()
    return _COMPILED


def kernel(x, wqkv, w_lin, b_lin, w_ff1, b_ff1, w_ff2, b_ff2):
    from concourse.bass_utils import run_bass_kernel_spmd

    x = np.asarray(x, np.float32)
    wqkv = np.asarray(wqkv, np.float32)
    Wq = wqkv[:, :D].astype(np.float64)
    Wk = wqkv[:, D:2 * D].astype(np.float64)
    Wv = wqkv[:, 2 * D:]

    wz = ((Wq @ Wk.T) / 2.0).astype(BF16)       # lhsT layout [a, d]:
    # y = qx @ wz.T folds QK^T into the query side; device contracts dim0
    wv8 = (Wv * 16.0).astype(FP8)               # fp8, x16 for dynamic range
    wlin = np.asarray(w_lin, np.float32).astype(BF16)
    wff1 = np.asarray(w_ff1, np.float32).astype(BF16)
    wff2 = np.asarray(w_ff2, np.float32).astype(BF16)
    masks = {p: _build_masks(p) for p in (0, 1)}

    in_maps = []
    qcols_by_parity = {
        0: np.r_[0:512, 1536:2048],
        1: np.r_[512:1536],
    }
    b_lin = np.asarray(b_lin, np.float32)
    b_ff1 = np.asarray(b_ff1, np.float32)
    b_ff2 = np.asarray(b_ff2, np.float32)
    bf1 = np.ascontiguousarray(b_ff1.reshape(ND, 128))
    for c in range(NCORES):
        b, h = c // 2, c % 2
        xT32 = np.ascontiguousarray(x[b].T)               # [D, S] f32
        qcols = qcols_by_parity[h]
        xqT = np.ascontiguousarray(xT32[:, qcols]) + b_lin[:, None]
        in_maps.append({
            "xT": xT32.astype(BF16),
            "xT8": xT32.astype(FP8),
            "xq": xqT.astype(F16),                        # b_lin folded in
            "wz": wz,
            "wv": wv8,
            "wlin": wlin,
            "wff1": wff1,
            "wff2": wff2,
            "masks": masks[h],
            "bf1": bf1,
            "par": np.full((1, 1), h, np.uint32),
        })

    global _LAST_IN_MAPS
    _LAST_IN_MAPS = in_maps
    nc = _get_program()
    res = run_bass_kernel_spmd(nc, in_maps, core_ids=list(range(NCORES)))

    out = np.empty((B, S, D), np.float32)
    for c in range(NCORES):
        b, h = c // 2, c % 2
        ol = res.results[c]["outT"].astype(np.float32).T  # [1024 s, D]
        if h == 0:
            out[b, 0:512] = ol[:512]
            out[b, 1536:2048] = ol[512:]
        else:
            out[b, 512:1536] = ol
    out += b_ff2[None, None, :]
    return out
